# revision 44
# baseline (speedup 1.0000x reference)
"""Trainium2 Bass kernel for nn_BiLSTMNet (2-layer BiLSTM + pair-gather MLP).

v2 design (latency-focused):
- Data-parallel across 8 cores (16 sentences each), fwd+bwd fused per step
  (32 rows) via block-diagonal K in the recurrent matmul.
- h^T lhsT tiles live in 16-step SBUF *history* buffers; DRAM h stores are
  batched to 4 DMAs per 16 steps (vs 8/step in v1) - kills the HWDGE
  descriptor-generation bottleneck (625ns per DMA instruction, single queue).
- bf16 elementwise chain (sigmoids/tanh outputs, cell state, products) for
  DVE 2x mode; fp32 only in PSUM matmul accumulators.
- x embedding gathers via Pool/SWDGE; x^T staging via PE transposes (no
  DMA-transposes).
- bwd-direction stores / loads use reversed-block access patterns so h0T/h1T
  hold time-aligned [hf(t); hb(t)] and every DMA is a single descriptor run.
- U = h1 @ w1^T phase batched (4-chunk groups); MLP pair-gather via SWDGE.
"""
import sys
sys.path.insert(0, "/opt/trn_rl_repo")
import numpy as np
import ml_dtypes

import concourse.bass as bass
import concourse.tile as tile
from concourse import mybir, bacc
from concourse.bass_utils import run_bass_kernel_spmd
from concourse.masks import make_identity

BF16 = mybir.dt.bfloat16
F32 = mybir.dt.float32
I32 = mybir.dt.int32
AF = mybir.ActivationFunctionType
ALU = mybir.AluOpType

V, E, H, B, C = 32000, 200, 200, 128, 256
T_FULL = 512
BL = 16            # sentences per core
NB = 32            # rows per step (16 fwd + 16 bwd)
NCORE = 8
CHT = 4            # steps per PSUM chunk (4*32 = 128 slots)
XG = 8             # steps per x-gather group (2 chunks)
NSTG = 16          # steps per store / D-load group
G4 = 800           # 4*H gate width


def build(T, n_cores, NPT):
    NSLOT = T * BL
    NCH = T // CHT
    NGX = T // XG
    NGS = T // NSTG

    nc = bacc.Bacc("TRN2", target_bir_lowering=False, debug=False,
                   enable_asserts=True, num_devices=n_cores)

    def din(name, shape, dt):
        return nc.dram_tensor(name, shape, dt, kind="ExternalInput").ap()

    def dout(name, shape, dt):
        return nc.dram_tensor(name, shape, dt, kind="ExternalOutput").ap()

    emb = din("emb", [V, E], BF16)
    W0s = din("W0s", [448, G4], BF16)      # x-proj chunks (128,96,128,96), bias@row95
    Whh0s = din("Whh0s", [400, G4], BF16)  # rec chunks (128,72,128,72)
    W1sf = din("W1sf", [401, G4], BF16)    # L1 x-proj fwd (128,128,128,17)
    W1sb = din("W1sb", [401, G4], BF16)
    Whh1s = din("Whh1s", [400, G4], BF16)
    WUs = din("WUs", [400, G4], BF16)      # U chunks (128,72,128,72)
    W2s = din("W2s", [512, 4], BF16)
    tokf = din("tokf", [128, NGX], I32)
    tokb = din("tokb", [128, NGX], I32)
    uidx0 = din("uidx0", [128, NPT], I32)
    uidx1 = din("uidx1", [128, NPT], I32)
    umask0 = din("umask0", [128, NPT], F32)
    umask1 = din("umask1", [128, NPT], F32)
    bw1m = din("bw1m", [128, 2 * H], BF16)

    OUT = dout("OUT", [NPT * 128, 4], F32)
    import os
    DEBUG = os.environ.get("KDEBUG", "") == "1"
    if DEBUG:
        DXF = dout("DXF", [128, 224], F32)
        DCF = dout("DCF", [128, 256], F32)
        DCB1 = dout("DCB1", [96, 256], F32)
        DXG = dout("DXG", [128, 400], F32)
        DHH = dout("DHH", [32, 224], F32)
        DA0 = dout("DA0", [128, 512], F32)
        DSG = dout("DSG", [32, 600], F32)
        DS = dout("DS", [32, 400], F32)
        DG0 = dout("DG0", [128, 400], F32)
        DSS = dout("DSS", [128, 400], F32)
        DHM = dout("DHM", [128, 512], F32)
        DEX = dout("DEX", [128, 4], F32)

    # internal DRAM: time-aligned h^T ([hf(t);hb(t)] at col block t)
    h0T = nc.dram_tensor("h0T", [401, NSLOT], BF16).ap()   # row 400 = ones
    h1T = nc.dram_tensor("h1T", [400, NSLOT], BF16).ap()
    U0d = nc.dram_tensor("U0d", [NSLOT, 2 * H], BF16).ap()
    U1d = nc.dram_tensor("U1d", [NSLOT, 2 * H], BF16).ap()

    RS0 = [(0, 128), (128, 224), (224, 352), (352, 448)]   # W0s chunks
    RSH = [(0, 128), (128, 200), (200, 328), (328, 400)]   # Whh/WU chunks
    RS1 = [(0, 128), (128, 256), (256, 384), (384, 401)]   # L1 x chunks

    with tile.TileContext(nc) as tc:
        with tc.tile_pool(name="const", bufs=1) as cp, \
             tc.tile_pool(name="state", bufs=1) as sp:

            def load_chunks(src, rowsets, ncols, pfx):
                tiles = []
                for i, (r0, r1) in enumerate(rowsets):
                    t_ = cp.tile([r1 - r0, ncols], BF16, tag=f"{pfx}{i}",
                                 name=f"{pfx}{i}")
                    nc.sync.dma_start(out=t_[:], in_=src[r0:r1, :])
                    tiles.append(t_)
                return tiles

            W0t = load_chunks(W0s, RS0, G4, "W0")
            Whh0t = load_chunks(Whh0s, RSH, G4, "Wh0")
            W1ft = load_chunks(W1sf, RS1, G4, "W1f")
            W1bt = load_chunks(W1sb, RS1, G4, "W1b")
            Whh1t = load_chunks(Whh1s, RSH, G4, "Wh1")
            WUt = load_chunks(WUs, RSH, G4, "WU")
            W2t = load_chunks(W2s, [(0, 128), (128, 256), (256, 384), (384, 512)],
                              4, "W2")

            tokf_t = cp.tile([128, NGX], I32)
            tokb_t = cp.tile([128, NGX], I32)
            nc.sync.dma_start(out=tokf_t[:], in_=tokf[:])
            nc.sync.dma_start(out=tokb_t[:], in_=tokb[:])

            ones_row = cp.tile([1, NSLOT], BF16)
            nc.vector.memset(ones_row[:], 1.0)
            nc.sync.dma_start(out=h0T[400:401, :], in_=ones_row[:])

            ident32 = sp.tile([32, 32], BF16, name="ident32")
            ident128 = sp.tile([128, 128], BF16, name="ident128")
            make_identity(nc, ident32[:])
            make_identity(nc, ident128[:])

            # ---- persistent state
            # h^T history buffers (16 step slots x 32 cols; f cols 0:16, b 16:32)
            Ah = [sp.tile([128, 32 * NSTG], BF16, tag="A0h", name="A0h"),
                  sp.tile([72, 32 * NSTG], BF16, tag="A1h", name="A1h"),
                  sp.tile([128, 32 * NSTG], BF16, tag="A2h", name="A2h"),
                  sp.tile([72, 32 * NSTG], BF16, tag="A3h", name="A3h")]
            S = sp.tile([NB, 2 * H], BF16, name="S")        # c | tanh(g)
            sigs = sp.tile([NB, 624], BF16, name="sigs")    # s(f),s(i) | s(o) | pad
            tcl = sp.tile([NB, 224], BF16, name="tcl")      # tanh(c) | pad
            prodt = sp.tile([NB, 2 * H], BF16, name="prodt")
            soT = sp.tile([128, 64], BF16, name="soT")      # s(o)^T staging
            tcT = sp.tile([128, 64], BF16, name="tcT")      # tanh(c)^T staging
            # x gather tiles + x^T lhsT (2-chunk groups, parity)
            xf = [sp.tile([128, 224], BF16, tag=f"xf{i}", name=f"xf{i}") for i in range(2)]
            xb = [sp.tile([128, 224], BF16, tag=f"xb{i}", name=f"xb{i}") for i in range(2)]
            Cf0 = [sp.tile([128, 256], BF16, tag=f"Cf0{i}", name=f"Cf0{i}") for i in range(2)]
            Cf1 = [sp.tile([96, 256], BF16, tag=f"Cf1{i}", name=f"Cf1{i}") for i in range(2)]
            Cb0 = [sp.tile([128, 256], BF16, tag=f"Cb0{i}", name=f"Cb0{i}") for i in range(2)]
            Cb1 = [sp.tile([96, 256], BF16, tag=f"Cb1{i}", name=f"Cb1{i}") for i in range(2)]
            # L1 lhsT tiles (4-chunk groups, parity); b-halves/f-halves stay 0
            Df = [[sp.tile([r1 - r0, 512], BF16, tag=f"Df{i}{j}", name=f"Df{i}{j}")
                   for i, (r0, r1) in enumerate(RS1)] for j in range(2)]
            Db = [[sp.tile([r1 - r0, 512], BF16, tag=f"Db{i}{j}", name=f"Db{i}{j}")
                   for i, (r0, r1) in enumerate(RS1)] for j in range(2)]

            for t_ in [S, sigs, tcl, prodt, soT, tcT] + Ah + xf + xb:
                nc.vector.memset(t_[:], 0.0)
            for j in range(2):
                for t_ in [Cf0[j], Cf1[j], Cb0[j], Cb1[j]] + Df[j] + Db[j]:
                    nc.vector.memset(t_[:], 0.0)
                # bias rows: 1.0 only in the direction's own column halves
                # (engine ops can't start at partition 95 -> use SBUF-SBUF DMA)
                ov = ones_row[:, 0:128].rearrange("p (a b) -> p a b", b=16)
                vf = Cf1[j][95:96, :].rearrange("p (a c b) -> p a c b", c=2, b=16)
                nc.sync.dma_start(out=vf[:, :, 0, :], in_=ov)
                vb = Cb1[j][95:96, :].rearrange("p (a c b) -> p a c b", c=2, b=16)
                nc.sync.dma_start(out=vb[:, :, 1, :], in_=ov)

            def ahf(i):
                # f-half view [p, s, 16] of history i
                return Ah[i][:].rearrange("p (s c b) -> p s c b", c=2, b=16)[:, :, 0, :]

            def ahb(i):
                return Ah[i][:].rearrange("p (s c b) -> p s c b", c=2, b=16)[:, :, 1, :]

            with tc.tile_pool(name="xps", bufs=2, space="PSUM") as xps, \
                 tc.tile_pool(name="tps", bufs=1, space="PSUM") as tps:

                # persistent PSUM staging tiles (bank-granular: pack regions)
                # psT: cols 0:32 tc^T lo | 32:64 tc^T hi | 64:96 so^T lo | 96:128 so^T hi
                psT = tps.tile([128, 128], BF16, tag="psT", name="psT")
                px = tps.tile([128, 256], BF16, tag="px", name="px")

                xg_tiles = {}

                def alloc_xg(k):
                    # fi / g / o in separate banks: each gate group start=True
                    # marks its whole 2KB zero region, so groups can't share
                    xgfi = xps.tile([128, 400], F32, space="PSUM", tag="xgfi",
                                    name="xgfi", padded_shape=[128, 512])
                    xgg = xps.tile([128, 200], F32, space="PSUM", tag="xgg",
                                   name="xgg", padded_shape=[128, 512])
                    xgo = xps.tile([128, 200], F32, space="PSUM", tag="xgo",
                                   name="xgo", padded_shape=[128, 512])
                    xg_tiles[k] = (xgfi, xgg, xgo)
                    return xg_tiles[k]

                def emit_xg0_mms(k, i):
                    # i-th matmul (of 12) of chunk k: fi kc0-3, g kc0-3, o kc0-3
                    xgfi, xgg, xgo = xg_tiles[k]
                    par = (k // 2) % 2
                    cc = k % 2
                    Ct = [Cf0[par], Cf1[par], Cb0[par], Cb1[par]]
                    kc = i % 4
                    lt = Ct[kc][:, 128 * cc:128 * cc + 128]
                    if i < 4:
                        nc.tensor.matmul(xgfi[:, 0:400], lt, W0t[kc][:, 0:400],
                                         start=(kc == 0), stop=(kc == 3))
                    elif i < 8:
                        nc.tensor.matmul(xgg[:, 0:200], lt, W0t[kc][:, 400:600],
                                         start=(kc == 0), stop=(kc == 3))
                    else:
                        nc.tensor.matmul(xgo[:, 0:200], lt, W0t[kc][:, 600:800],
                                         start=(kc == 0), stop=(kc == 3))

                def emit_xg1_mms(k, i):
                    # i-th matmul (of 24) of chunk k: fi kc0-7, g kc0-7, o kc0-7
                    xgfi, xgg, xgo = xg_tiles[k]
                    par = (k // 4) % 2
                    cc = k % 4
                    Dt = Df[par] + Db[par]
                    kc = i % 8
                    lt = Dt[kc][:, 128 * cc:128 * cc + 128]
                    if i < 8:
                        nc.tensor.matmul(xgfi[:, 0:400], lt,
                                         (W1ft + W1bt)[kc][:, 0:400],
                                         start=(kc == 0), stop=(kc == 7))
                    elif i < 16:
                        nc.tensor.matmul(xgg[:, 0:200], lt,
                                         (W1ft + W1bt)[kc][:, 400:600],
                                         start=(kc == 0), stop=(kc == 7))
                    else:
                        nc.tensor.matmul(xgo[:, 0:200], lt,
                                         (W1ft + W1bt)[kc][:, 600:800],
                                         start=(kc == 0), stop=(kc == 7))

                def emit_gathers(g):
                    par = g % 2
                    nc.gpsimd.indirect_dma_start(
                        out=xf[par][:, 0:E], out_offset=None, in_=emb[:],
                        in_offset=bass.IndirectOffsetOnAxis(ap=tokf_t[:, g:g + 1], axis=0))
                    nc.gpsimd.indirect_dma_start(
                        out=xb[par][:, 0:E], out_offset=None, in_=emb[:],
                        in_offset=bass.IndirectOffsetOnAxis(ap=tokb_t[:, g:g + 1], axis=0))

                def emit_xpose(g):
                    # PE transposes + strided copies into C tiles for group g
                    par = g % 2
                    for (src, C0, C1, dve) in ((xf[par], Cf0[par], Cf1[par], True),
                                               (xb[par], Cb0[par], Cb1[par], False)):
                        half = 0 if dve else 1
                        nc.tensor.transpose(px[:, 0:128], src[:, 0:128], ident128[:])
                        nc.tensor.transpose(px[0:96, 128:256], src[:, 128:224],
                                            ident128[:])
                        d0 = C0[:].rearrange("p (a c b) -> p a c b", c=2, b=16)[:, :, half, :]
                        s0 = px[:, 0:128].rearrange("p (a b) -> p a b", b=16)
                        d1 = C1[0:72].rearrange("p (a c b) -> p a c b", c=2, b=16)[:, :, half, :]
                        s1 = px[0:72, 128:256].rearrange("p (a b) -> p a b", b=16)
                        if dve:
                            nc.vector.tensor_copy(d0, s0)
                            nc.scalar.copy(d1, s1)
                        else:
                            nc.scalar.copy(d0, s0)
                            nc.vector.tensor_copy(d1, s1)

                def emit_step(p, Whht):
                    k = p // CHT
                    r = (p % CHT) * NB
                    sp_ = (p - 1) % NSTG
                    xgfi, xgg, xgo = xg_tiles[k]
                    # recurrent matmuls (accumulate onto x-proj); kc order
                    # 0,2,1,3 matches the h^T mul emission order below
                    KCO = (0, 2, 1, 3)
                    for (t_, w0, w1) in ((xgfi, 0, 400), (xgg, 400, 600),
                                         (xgo, 600, 800)):
                        for ei, kc in enumerate(KCO):
                            nc.tensor.matmul(t_[r:r + NB, 0:w1 - w0],
                                             Ah[kc][:, 32 * sp_:32 * sp_ + 32],
                                             Whht[kc][:, w0:w1],
                                             start=False, stop=(ei == 3),
                                             skip_group_check=True,
                                             tile_position=(0, r))
                    # gate nonlinearities
                    nc.scalar.activation(sigs[:, 0:400], xgfi[r:r + NB, 0:400],
                                         AF.Sigmoid)
                    nc.scalar.activation(S[:, H:2 * H], xgg[r:r + NB, 0:200],
                                         AF.Tanh)
                    nc.scalar.activation(sigs[:, 400:600], xgo[r:r + NB, 0:200],
                                         AF.Sigmoid)
                    # c update
                    nc.vector.tensor_mul(prodt[:], sigs[:, 0:400], S[:])
                    nc.vector.tensor_add(S[:, 0:H], prodt[:, 0:H], prodt[:, H:2 * H])
                    nc.scalar.activation(tcl[:, 0:H], S[:, 0:H], AF.Tanh)

                def emit_step_tail(p):
                    # h^T = so^T * tc^T computed directly in transposed space:
                    # PE transposes so (early, off-chain) and tc; DVE muls
                    # write the history slots (block-diag halves)
                    sn = p % NSTG
                    nc.tensor.transpose(psT[:, 64:96], sigs[:, 400:528], ident32[:])
                    nc.tensor.transpose(psT[0:96, 96:128], sigs[:, 528:624],
                                        ident32[:])
                    nc.tensor.transpose(psT[:, 0:32], tcl[:, 0:128], ident32[:])
                    nc.tensor.transpose(psT[0:96, 32:64], tcl[:, 128:224],
                                        ident32[:])
                    # stage so^T (Act, off-chain) and tc^T (DVE) to SBUF, then
                    # all-SBUF muls write the history slots (68ns each on DVE)
                    nc.scalar.copy(soT[:, 0:32], psT[:, 64:96])
                    nc.scalar.copy(soT[0:72, 32:64], psT[0:72, 96:128])
                    nc.vector.tensor_copy(tcT[:, 0:32], psT[:, 0:32])
                    nc.vector.tensor_mul(Ah[0][:, 32 * sn:32 * sn + 16],
                                         tcT[:, 0:16], soT[:, 0:16])
                    nc.vector.tensor_mul(Ah[2][:, 32 * sn + 16:32 * sn + 32],
                                         tcT[:, 16:32], soT[:, 16:32])
                    nc.vector.tensor_copy(tcT[0:72, 32:64], psT[0:72, 32:64])
                    nc.vector.tensor_mul(Ah[1][:, 32 * sn:32 * sn + 16],
                                         tcT[0:72, 32:48], soT[0:72, 32:48])
                    nc.vector.tensor_mul(Ah[3][:, 32 * sn + 16:32 * sn + 32],
                                         tcT[0:72, 48:64], soT[0:72, 48:64])

                def emit_stores(G, hT, T_):
                    # batched h^T stores for 16-step group G
                    c0 = G * NSTG * BL
                    cr0 = (T_ - NSTG * (G + 1)) * BL
                    hTv = hT[:].rearrange("p (t b) -> p t b", b=16)
                    nc.sync.dma_start(out=hTv[0:128, G * NSTG:(G + 1) * NSTG, :],
                                      in_=ahf(0))
                    nc.sync.dma_start(out=hTv[128:200, G * NSTG:(G + 1) * NSTG, :],
                                      in_=ahf(1))
                    rb = T_ // NSTG - 1 - G
                    nc.sync.dma_start(out=hTv[200:328, rb * NSTG:(rb + 1) * NSTG, :],
                                      in_=ahb(2)[:, ::-1, :])
                    nc.sync.dma_start(out=hTv[328:400, rb * NSTG:(rb + 1) * NSTG, :],
                                      in_=ahb(3)[:, ::-1, :])

                def emit_dloads(G):
                    # L1 lhsT loads for 16-step group G (4 chunks)
                    par = G % 2
                    for ri, (r0, r1) in enumerate(RS1):
                        dst = Df[par][ri][:].rearrange(
                            "p (a c b) -> p a c b", c=2, b=16)[:, :, 0, :]
                        src = h0T[r0:r1, G * 256:(G + 1) * 256].rearrange(
                            "p (s b) -> p s b", b=16)
                        nc.sync.dma_start(out=dst, in_=src)
                    rb = NGS - 1 - G
                    for ri, (r0, r1) in enumerate(RS1):
                        dst = Db[par][ri][:].rearrange(
                            "p (a c b) -> p a c b", c=2, b=16)[:, :, 1, :]
                        src = h0T[r0:r1, rb * 256:(rb + 1) * 256].rearrange(
                            "p (s b) -> p s b", b=16)[:, ::-1, :]
                        nc.sync.dma_start(out=dst, in_=src)

                def reset_states():
                    nc.vector.memset(S[:], 0.0)
                    for t_ in Ah:
                        nc.vector.memset(t_[:], 0.0)

                # ================= layer 0 =================
                emit_gathers(0)
                emit_xpose(0)
                alloc_xg(0)
                for i in range(12):
                    emit_xg0_mms(0, i)
                if DEBUG:
                    dstg = sp.tile([128, 400], F32, name="dstg")
                    dstg2 = sp.tile([128, 224], F32, name="dstg2")
                    nc.vector.tensor_copy(dstg2[:], xf[0][:])
                    nc.sync.dma_start(out=DXF[:], in_=dstg2[:])
                    nc.vector.tensor_copy(dstg[:, 0:256], Cf0[0][:])
                    nc.sync.dma_start(out=DCF[:], in_=dstg[:, 0:256])
                    nc.vector.tensor_copy(dstg[0:96, 0:256], Cb1[0][:])
                    nc.sync.dma_start(out=DCB1[:], in_=dstg[0:96, 0:256])
                    nc.vector.tensor_copy(dstg[:], xg_tiles[0][0][:])
                    nc.sync.dma_start(out=DXG[:], in_=dstg[:])
                for G2 in range(NGX):
                    if G2 + 1 < NGX:
                        emit_gathers(G2 + 1)
                    for a in range(XG):
                        p = XG * G2 + a
                        emit_step(p, Whh0t)
                        if G2 + 1 < NGX and a == 2:
                            emit_xpose(G2 + 1)
                        k = p // CHT
                        if k + 1 < NCH:
                            if a % CHT == 0:
                                alloc_xg(k + 1)
                            for q in range(3):
                                emit_xg0_mms(k + 1, (a % CHT) * 3 + q)
                        emit_step_tail(p)
                        if a % CHT == CHT - 1:
                            xg_tiles.pop(k, None)
                        if p % NSTG == NSTG - 1:
                            emit_stores(p // NSTG, h0T, T)
                # ================= layer 1 =================
                reset_states()
                emit_dloads(0)
                alloc_xg(0)
                for i in range(24):
                    emit_xg1_mms(0, i)
                for G in range(NGS):
                    if G + 1 < NGS:
                        emit_dloads(G + 1)
                    for a in range(NSTG):
                        p = NSTG * G + a
                        emit_step(p, Whh1t)
                        k = p // CHT
                        if k + 1 < NCH:
                            if a % CHT == 0:
                                alloc_xg(k + 1)
                            i0 = (a % CHT) * 6
                            for i in range(i0, i0 + 6):
                                emit_xg1_mms(k + 1, i)
                        emit_step_tail(p)
                        if a % CHT == CHT - 1:
                            xg_tiles.pop(k, None)
                    emit_stores(G, h1T, T)

            # ================= U phase =================
            with tc.tile_pool(name="uw", bufs=2) as uw, \
                 tc.tile_pool(name="ups", bufs=2, space="PSUM") as ups:
                for G in range(NSLOT // 512):
                    ut = []
                    for ri, (r0, r1) in enumerate(RSH):
                        t_ = uw.tile([r1 - r0, 512], BF16, tag=f"UL{ri}",
                                     name=f"UL{ri}")
                        nc.sync.dma_start(out=t_[:],
                                          in_=h1T[r0:r1, 512 * G:512 * (G + 1)])
                        ut.append(t_)
                    ustg = uw.tile([128, 3200], BF16, tag="ustg", name="ustg")
                    for cc in range(4):
                        psu0 = ups.tile([128, 400], F32, space="PSUM", tag="psu0",
                                        name="psu0", padded_shape=[128, 512])
                        psu1 = ups.tile([128, 400], F32, space="PSUM", tag="psu1",
                                        name="psu1", padded_shape=[128, 512])
                        for kc in range(4):
                            lt = ut[kc][:, 128 * cc:128 * cc + 128]
                            nc.tensor.matmul(psu0[:], lt, WUt[kc][:, 0:400],
                                             start=(kc == 0), stop=(kc == 3))
                            nc.tensor.matmul(psu1[:], lt, WUt[kc][:, 400:800],
                                             start=(kc == 0), stop=(kc == 3))
                        nc.vector.tensor_copy(ustg[:, 800 * cc:800 * cc + 400],
                                              psu0[:])
                        nc.scalar.copy(ustg[:, 800 * cc + 400:800 * (cc + 1)],
                                       psu1[:])
                    sv = ustg[:].rearrange("p (c j) -> p c j", j=800)
                    d0 = U0d[512 * G:512 * (G + 1), :].rearrange(
                        "(c p) j -> p c j", p=128)
                    d1 = U1d[512 * G:512 * (G + 1), :].rearrange(
                        "(c p) j -> p c j", p=128)
                    nc.sync.dma_start(out=d0, in_=sv[:, :, 0:400])
                    nc.sync.dma_start(out=d1, in_=sv[:, :, 400:800])

            # ================= gather + MLP =================
            with tc.tile_pool(name="fw", bufs=2) as fw, \
                 tc.tile_pool(name="fc", bufs=1) as fc, \
                 tc.tile_pool(name="fps", bufs=2, space="PSUM") as fps:
                ui0 = fc.tile([128, NPT], I32)
                ui1 = fc.tile([128, NPT], I32)
                um0 = fc.tile([128, NPT], F32)
                um1 = fc.tile([128, NPT], F32)
                nc.sync.dma_start(out=ui0[:], in_=uidx0[:])
                nc.sync.dma_start(out=ui1[:], in_=uidx1[:])
                nc.sync.dma_start(out=um0[:], in_=umask0[:])
                nc.sync.dma_start(out=um1[:], in_=umask1[:])
                bwt = fc.tile([128, 2 * H], BF16, name="bwt")
                nc.sync.dma_start(out=bwt[:], in_=bw1m[:])
                hm = [fc.tile([128, 512], BF16, tag=f"hm{i}", name=f"hm{i}")
                      for i in range(2)]
                for t_ in hm:
                    nc.vector.memset(t_[:], 0.0)
                    nc.vector.memset(t_[:, 511:512], 1.0)
                otstg = fc.tile([128, 16], F32, name="otstg")
                for j in range(NPT):
                    par = j % 2
                    g0 = fw.tile([128, 2 * H], BF16, tag="g0", name="g0")
                    g1 = fw.tile([128, 2 * H], BF16, tag="g1", name="g1")
                    nc.gpsimd.indirect_dma_start(
                        out=g0[:], out_offset=None, in_=U0d[:],
                        in_offset=bass.IndirectOffsetOnAxis(ap=ui0[:, j:j + 1], axis=0))
                    nc.gpsimd.indirect_dma_start(
                        out=g1[:], out_offset=None, in_=U1d[:],
                        in_offset=bass.IndirectOffsetOnAxis(ap=ui1[:, j:j + 1], axis=0))
                    g1m = fw.tile([128, 2 * H], BF16, tag="g1m", name="g1m")
                    nc.vector.scalar_tensor_tensor(g1m[:], g1[:], um1[:, j:j + 1],
                                                   bwt[:], ALU.mult, ALU.add)
                    ssum = fw.tile([128, 2 * H], BF16, tag="ssum", name="ssum")
                    nc.vector.scalar_tensor_tensor(ssum[:], g0[:], um0[:, j:j + 1],
                                                   g1m[:], ALU.mult, ALU.add)
                    nc.scalar.activation(hm[par][:, 0:2 * H], ssum[:], AF.Tanh)
                    psl = fps.tile([128, 4], F32, space="PSUM", tag="psl", name="psl")
                    pst = fps.tile([128, 512], BF16, space="PSUM", tag="pst",
                                   name="pst")
                    for i in range(4):
                        nc.tensor.transpose(pst[:, 128 * i:128 * (i + 1)],
                                            hm[par][:, 128 * i:128 * (i + 1)],
                                            ident128[:])
                        hmT = fw.tile([128, 128], BF16, tag=f"hmT{i}", name=f"hmT{i}")
                        if i % 2 == 0:
                            nc.vector.tensor_copy(hmT[:], pst[:, 128 * i:128 * (i + 1)])
                        else:
                            nc.scalar.copy(hmT[:], pst[:, 128 * i:128 * (i + 1)])
                        nc.tensor.matmul(psl[:], hmT[:], W2t[i][:],
                                         start=(i == 0), stop=(i == 3))
                    ex = fw.tile([128, 4], F32, tag="ex", name="ex")
                    nc.scalar.activation(ex[:], psl[:], AF.Exp)
                    if DEBUG and j == 0:
                        dmt = fc.tile([128, 512], F32, name="dmt")
                        nc.vector.tensor_copy(dmt[:, 0:400], g0[:])
                        nc.sync.dma_start(out=DG0[:], in_=dmt[:, 0:400])
                        nc.vector.tensor_copy(dmt[:, 0:400], ssum[:])
                        nc.sync.dma_start(out=DSS[:], in_=dmt[:, 0:400])
                        nc.vector.tensor_copy(dmt[:, 0:512], hm[par][:])
                        nc.sync.dma_start(out=DHM[:], in_=dmt[:, 0:512])
                        nc.vector.tensor_copy(dmt[:, 0:4], ex[:])
                        nc.sync.dma_start(out=DEX[:], in_=dmt[:, 0:4])
                    sm = fw.tile([128, 1], F32, tag="sm", name="sm")
                    nc.vector.reduce_sum(sm[:], ex[:], axis=mybir.AxisListType.X)
                    rc = fw.tile([128, 1], F32, tag="rc", name="rc")
                    nc.vector.reciprocal(rc[:], sm[:])
                    jj = j % 4
                    nc.vector.tensor_scalar_mul(otstg[:, 4 * jj:4 * jj + 4],
                                                ex[:], rc[:, 0:1])
                    if DEBUG and j == 0:
                        dmt2 = fc.tile([128, 8], F32, name="dmt2")
                        nc.vector.tensor_copy(dmt2[:, 0:1], sm[:])
                        nc.vector.tensor_copy(dmt2[:, 1:2], rc[:])
                        nc.vector.tensor_copy(dmt2[:, 2:6], otstg[:, 0:4])
                        nc.sync.dma_start(out=DEX[:], in_=dmt2[:, 2:6])
                    if jj == 3 or j == NPT - 1:
                        nb = jj + 1
                        dst = OUT[128 * (j - jj):128 * (j + 1), :].rearrange(
                            "(c p) j -> p c j", p=128)
                        src = otstg[:, 0:4 * nb].rearrange("p (c j) -> p c j", j=4)
                        nc.sync.dma_start(out=dst, in_=src)
    nc.compile()
    return nc


# ---------------------------------------------------------------------------
# host-side preparation
# ---------------------------------------------------------------------------

def _perm_gates(w):
    """torch gate order (i,f,g,o) -> (f,i,g,o) along axis 0 (4H rows)."""
    Hq = w.shape[0] // 4
    i, f, g, o = (w[0:Hq], w[Hq:2 * Hq], w[2 * Hq:3 * Hq], w[3 * Hq:4 * Hq])
    return np.concatenate([f, i, g, o], axis=0)


def prepare_inputs(inputs, T, n_cores):
    bf = ml_dtypes.bfloat16
    C_ = np.asarray(inputs["confs"]).shape[1]
    emb = np.asarray(inputs["emb"], np.float32)
    tokens = np.asarray(inputs["tokens"])
    confs = np.asarray(inputs["confs"])

    p = {}
    p["emb"] = emb.astype(bf)

    def gp(name):
        return _perm_gates(np.asarray(inputs[name], np.float32))

    Wih0f, Wih0b = gp("Wih0f"), gp("Wih0b")
    b0f, b0b = gp("b0f"), gp("b0b")
    Whh0f, Whh0b = gp("Whh0f"), gp("Whh0b")
    Wih1f, Wih1b = gp("Wih1f"), gp("Wih1b")
    b1f, b1b = gp("b1f"), gp("b1b")
    Whh1f, Whh1b = gp("Whh1f"), gp("Whh1b")
    w1 = np.asarray(inputs["w1"], np.float32)
    bw1 = np.asarray(inputs["bw1"], np.float32)
    w2 = np.asarray(inputs["w2"], np.float32)
    bw2 = np.asarray(inputs["bw2"], np.float32)

    # W0s: x-proj chunks (128, 96, 128, 96); bias at row 95 of 96-chunks
    w0 = np.zeros((448, G4), np.float32)
    w0[0:128] = Wih0f.T[0:128]
    w0[128:200] = Wih0f.T[128:200]
    w0[223] = b0f
    w0[224:352] = Wih0b.T[0:128]
    w0[352:424] = Wih0b.T[128:200]
    w0[447] = b0b
    p["W0s"] = w0.astype(bf)

    def rec_stream(wf, wb):
        o = np.zeros((400, G4), np.float32)
        o[0:128] = wf.T[0:128]
        o[128:200] = wf.T[128:200]
        o[200:328] = wb.T[0:128]
        o[328:400] = wb.T[128:200]
        return o.astype(bf)

    p["Whh0s"] = rec_stream(Whh0f, Whh0b)
    p["Whh1s"] = rec_stream(Whh1f, Whh1b)
    p["W1sf"] = np.concatenate([Wih1f.T, b1f[None, :]], 0).astype(bf)
    p["W1sb"] = np.concatenate([Wih1b.T, b1b[None, :]], 0).astype(bf)

    # WUs: K = h1 feature, N = [U0 cols | U1 cols]
    w1T = w1.T  # [800, 400]
    wu = np.zeros((400, G4), np.float32)
    wu[:, 0:400] = w1T[0:400]
    wu[:, 400:800] = w1T[400:800]
    p["WUs"] = wu.astype(bf)
    p["bw1m"] = np.tile(bw1[None, :], (128, 1)).astype(bf)
    w2p = np.zeros((512, 4), np.float32)
    w2p[0:2 * H] = w2.T
    w2p[511] = bw2
    p["W2s"] = w2p.astype(bf)

    NGX = T // XG
    NP = BL * C_
    NPT = (NP + 127) // 128

    in_maps = []
    for c in range(n_cores):
        m = dict(p)
        bs = tokens[c * BL:(c + 1) * BL, 0:T]          # [BL, T]
        tf = np.zeros((128, NGX), np.int32)
        tb = np.zeros((128, NGX), np.int32)
        for g in range(NGX):
            for a in range(XG):
                tf[a * BL:(a + 1) * BL, g] = bs[:, XG * g + a]
                tb[a * BL:(a + 1) * BL, g] = bs[:, T - 1 - (XG * g + a)]
        m["tokf"] = tf
        m["tokb"] = tb
        cf = confs[c * BL:(c + 1) * BL]                 # [BL, C, 2]
        t0 = cf[:, :, 0].reshape(-1)                    # row-major (b, ci)
        t1 = cf[:, :, 1].reshape(-1)
        bidx = np.repeat(np.arange(BL), C_)
        ui0 = np.clip(t0, 0, T - 1) * BL + bidx
        ui1 = np.clip(t1, 0, T - 1) * BL + bidx
        um0 = (t0 >= 0).astype(np.float32)
        um1 = (t1 >= 0).astype(np.float32)

        def tile128(a, dt):
            o = np.zeros((NPT * 128,), dt)
            o[:a.shape[0]] = a
            return o.reshape(NPT, 128).T.copy()
        m["uidx0"] = tile128(ui0.astype(np.int32), np.int32)
        m["uidx1"] = tile128(ui1.astype(np.int32), np.int32)
        m["umask0"] = tile128(um0, np.float32)
        m["umask1"] = tile128(um1, np.float32)
        in_maps.append(m)
    return in_maps


_CACHE = {}


def _get_prog(T, n_cores, NPT):
    key = (T, n_cores, NPT)
    if key not in _CACHE:
        _CACHE[key] = build(T, n_cores, NPT)
    return _CACHE[key]


def kernel(**inputs):
    T = inputs["tokens"].shape[1]
    C_ = inputs["confs"].shape[1]
    n_cores = NCORE
    NP = BL * C_
    NPT = (NP + 127) // 128
    nc = _get_prog(T, n_cores, NPT)
    in_maps = prepare_inputs(inputs, T, n_cores)
    res = run_bass_kernel_spmd(nc, in_maps, list(range(n_cores)))
    outs = []
    for c in range(n_cores):
        o = res.results[c]["OUT"][:NP]          # [BL*C, 4] rows (b, ci)
        outs.append(o)
    return np.concatenate(outs, axis=0).astype(np.float32)


# revision 48
# speedup vs baseline: 1.0838x; 1.0838x over previous
"""Trainium2 Bass kernel for nn_BiLSTMNet (2-layer BiLSTM + pair-gather MLP).

v2 design (latency-focused):
- Data-parallel across 8 cores (16 sentences each), fwd+bwd fused per step
  (32 rows) via block-diagonal K in the recurrent matmul.
- h^T lhsT tiles live in 16-step SBUF *history* buffers; DRAM h stores are
  batched to 4 DMAs per 16 steps (vs 8/step in v1) - kills the HWDGE
  descriptor-generation bottleneck (625ns per DMA instruction, single queue).
- bf16 elementwise chain (sigmoids/tanh outputs, cell state, products) for
  DVE 2x mode; fp32 only in PSUM matmul accumulators.
- x embedding gathers via Pool/SWDGE; x^T staging via PE transposes (no
  DMA-transposes).
- bwd-direction stores / loads use reversed-block access patterns so h0T/h1T
  hold time-aligned [hf(t); hb(t)] and every DMA is a single descriptor run.
- U = h1 @ w1^T phase batched (4-chunk groups); MLP pair-gather via SWDGE.
"""
import sys
sys.path.insert(0, "/opt/trn_rl_repo")
import numpy as np
import ml_dtypes

import concourse.bass as bass
import concourse.tile as tile
from concourse import mybir, bacc
from concourse.bass_utils import run_bass_kernel_spmd
from concourse.masks import make_identity

BF16 = mybir.dt.bfloat16
F32 = mybir.dt.float32
I32 = mybir.dt.int32
AF = mybir.ActivationFunctionType
ALU = mybir.AluOpType

V, E, H, B, C = 32000, 200, 200, 128, 256
T_FULL = 512
BL = 16            # sentences per core
NB = 32            # rows per step (16 fwd + 16 bwd)
NCORE = 8
CHT = 4            # steps per PSUM chunk (4*32 = 128 slots)
XG = 8             # steps per x-gather group (2 chunks)
NSTG = 16          # steps per store / D-load group
G4 = 800           # 4*H gate width


def build(T, n_cores, NPT):
    NSLOT = T * BL
    NCH = T // CHT
    NGX = T // XG
    NGS = T // NSTG

    nc = bacc.Bacc("TRN2", target_bir_lowering=False, debug=False,
                   enable_asserts=True, num_devices=n_cores)

    def din(name, shape, dt):
        return nc.dram_tensor(name, shape, dt, kind="ExternalInput").ap()

    def dout(name, shape, dt):
        return nc.dram_tensor(name, shape, dt, kind="ExternalOutput").ap()

    emb = din("emb", [V, E], BF16)
    W0s = din("W0s", [448, G4], BF16)      # x-proj chunks (128,96,128,96), bias@row95
    Whh0s = din("Whh0s", [400, G4], BF16)  # rec chunks (128,72,128,72)
    W1sf = din("W1sf", [401, G4], BF16)    # L1 x-proj fwd (128,128,128,17)
    W1sb = din("W1sb", [401, G4], BF16)
    Whh1s = din("Whh1s", [400, G4], BF16)
    WUs = din("WUs", [400, G4], BF16)      # U chunks (128,72,128,72)
    W2s = din("W2s", [512, 4], BF16)
    tokf = din("tokf", [128, NGX], I32)
    tokb = din("tokb", [128, NGX], I32)
    uidx0 = din("uidx0", [128, NPT], I32)
    uidx1 = din("uidx1", [128, NPT], I32)
    umask0 = din("umask0", [128, NPT], F32)
    umask1 = din("umask1", [128, NPT], F32)
    bw1m = din("bw1m", [128, 2 * H], BF16)

    OUT = dout("OUT", [NPT * 128, 4], F32)
    import os
    DEBUG = os.environ.get("KDEBUG", "") == "1"
    if DEBUG:
        DXF = dout("DXF", [128, 224], F32)
        DCF = dout("DCF", [128, 256], F32)
        DCB1 = dout("DCB1", [96, 256], F32)
        DXG = dout("DXG", [128, 400], F32)
        DHH = dout("DHH", [32, 224], F32)
        DA0 = dout("DA0", [128, 512], F32)
        DSG = dout("DSG", [32, 600], F32)
        DS = dout("DS", [32, 400], F32)
        DG0 = dout("DG0", [128, 400], F32)
        DSS = dout("DSS", [128, 400], F32)
        DHM = dout("DHM", [128, 512], F32)
        DEX = dout("DEX", [128, 4], F32)

    # internal DRAM: time-aligned h^T ([hf(t);hb(t)] at col block t)
    h0T = nc.dram_tensor("h0T", [401, NSLOT], BF16).ap()   # row 400 = ones
    h1T = nc.dram_tensor("h1T", [400, NSLOT], BF16).ap()
    U0d = nc.dram_tensor("U0d", [NSLOT, 2 * H], BF16).ap()
    U1d = nc.dram_tensor("U1d", [NSLOT, 2 * H], BF16).ap()

    RS0 = [(0, 128), (128, 224), (224, 352), (352, 448)]   # W0s chunks
    RSH = [(0, 128), (128, 200), (200, 328), (328, 400)]   # Whh/WU chunks
    RS1 = [(0, 128), (128, 256), (256, 384), (384, 401)]   # L1 x chunks

    with tile.TileContext(nc) as tc:
        with tc.tile_pool(name="const", bufs=1) as cp, \
             tc.tile_pool(name="state", bufs=1) as sp:

            def load_chunks(src, rowsets, ncols, pfx):
                tiles = []
                for i, (r0, r1) in enumerate(rowsets):
                    t_ = cp.tile([r1 - r0, ncols], BF16, tag=f"{pfx}{i}",
                                 name=f"{pfx}{i}")
                    nc.sync.dma_start(out=t_[:], in_=src[r0:r1, :])
                    tiles.append(t_)
                return tiles

            W0t = load_chunks(W0s, RS0, G4, "W0")
            Whh0t = load_chunks(Whh0s, RSH, G4, "Wh0")
            W1ft = load_chunks(W1sf, RS1, G4, "W1f")
            W1bt = load_chunks(W1sb, RS1, G4, "W1b")
            Whh1t = load_chunks(Whh1s, RSH, G4, "Wh1")
            WUt = load_chunks(WUs, RSH, G4, "WU")
            W2t = load_chunks(W2s, [(0, 128), (128, 256), (256, 384), (384, 512)],
                              4, "W2")

            tokf_t = cp.tile([128, NGX], I32)
            tokb_t = cp.tile([128, NGX], I32)
            nc.sync.dma_start(out=tokf_t[:], in_=tokf[:])
            nc.sync.dma_start(out=tokb_t[:], in_=tokb[:])

            ones_row = cp.tile([1, NSLOT], BF16)
            nc.vector.memset(ones_row[:], 1.0)
            nc.sync.dma_start(out=h0T[400:401, :], in_=ones_row[:])

            ident32 = sp.tile([32, 32], BF16, name="ident32")
            ident128 = sp.tile([128, 128], BF16, name="ident128")
            make_identity(nc, ident32[:])
            make_identity(nc, ident128[:])

            # ---- persistent state
            # h^T history buffers (16 step slots x 32 cols; f cols 0:16, b 16:32)
            Ah = [sp.tile([128, 32 * NSTG], BF16, tag="A0h", name="A0h"),
                  sp.tile([72, 32 * NSTG], BF16, tag="A1h", name="A1h"),
                  sp.tile([128, 32 * NSTG], BF16, tag="A2h", name="A2h"),
                  sp.tile([72, 32 * NSTG], BF16, tag="A3h", name="A3h")]
            S = sp.tile([NB, 2 * H], BF16, name="S")        # c | tanh(g)
            sigs = sp.tile([NB, 624], BF16, name="sigs")    # s(f),s(i) | s(o) | pad
            tcl = sp.tile([NB, 224], BF16, name="tcl")      # tanh(c) | pad
            prodt = sp.tile([NB, 2 * H], BF16, name="prodt")
            soT = sp.tile([128, 64], BF16, name="soT")      # s(o)^T staging
            # x gather tiles + x^T lhsT (2-chunk groups, parity)
            xf = [sp.tile([128, 224], BF16, tag=f"xf{i}", name=f"xf{i}") for i in range(2)]
            xb = [sp.tile([128, 224], BF16, tag=f"xb{i}", name=f"xb{i}") for i in range(2)]
            Cf0 = [sp.tile([128, 256], BF16, tag=f"Cf0{i}", name=f"Cf0{i}") for i in range(2)]
            Cf1 = [sp.tile([96, 256], BF16, tag=f"Cf1{i}", name=f"Cf1{i}") for i in range(2)]
            Cb0 = [sp.tile([128, 256], BF16, tag=f"Cb0{i}", name=f"Cb0{i}") for i in range(2)]
            Cb1 = [sp.tile([96, 256], BF16, tag=f"Cb1{i}", name=f"Cb1{i}") for i in range(2)]
            # L1 lhsT tiles (4-chunk groups, parity); b-halves/f-halves stay 0
            Df = [[sp.tile([r1 - r0, 512], BF16, tag=f"Df{i}{j}", name=f"Df{i}{j}")
                   for i, (r0, r1) in enumerate(RS1)] for j in range(2)]
            Db = [[sp.tile([r1 - r0, 512], BF16, tag=f"Db{i}{j}", name=f"Db{i}{j}")
                   for i, (r0, r1) in enumerate(RS1)] for j in range(2)]

            for t_ in [S, sigs, tcl, prodt, soT] + Ah + xf + xb:
                nc.vector.memset(t_[:], 0.0)
            for j in range(2):
                for t_ in [Cf0[j], Cf1[j], Cb0[j], Cb1[j]] + Df[j] + Db[j]:
                    nc.vector.memset(t_[:], 0.0)
                # bias rows: 1.0 only in the direction's own column halves
                # (engine ops can't start at partition 95 -> use SBUF-SBUF DMA)
                ov = ones_row[:, 0:128].rearrange("p (a b) -> p a b", b=16)
                vf = Cf1[j][95:96, :].rearrange("p (a c b) -> p a c b", c=2, b=16)
                nc.sync.dma_start(out=vf[:, :, 0, :], in_=ov)
                vb = Cb1[j][95:96, :].rearrange("p (a c b) -> p a c b", c=2, b=16)
                nc.sync.dma_start(out=vb[:, :, 1, :], in_=ov)

            def ahf(i):
                # f-half view [p, s, 16] of history i
                return Ah[i][:].rearrange("p (s c b) -> p s c b", c=2, b=16)[:, :, 0, :]

            def ahb(i):
                return Ah[i][:].rearrange("p (s c b) -> p s c b", c=2, b=16)[:, :, 1, :]

            with tc.tile_pool(name="xps", bufs=2, space="PSUM") as xps, \
                 tc.tile_pool(name="tps", bufs=1, space="PSUM") as tps:

                # persistent PSUM staging tiles. Zero-region (bank) granularity
                # matters: a transpose's start=True marks its whole bank, so
                # readers of OTHER regions in that bank get serialized after
                # it. tc^T gets its own bank; so^T shares the x-stage bank
                # (written only once per 8 steps).
                psTc = tps.tile([128, 64], BF16, tag="psTc", name="psTc")
                px = tps.tile([128, 320], BF16, tag="px", name="px")

                xg_tiles = {}

                def alloc_xg(k):
                    # fi / g / o in separate banks: each gate group start=True
                    # marks its whole 2KB zero region, so groups can't share
                    xgfi = xps.tile([128, 400], F32, space="PSUM", tag="xgfi",
                                    name="xgfi", padded_shape=[128, 512])
                    xgg = xps.tile([128, 200], F32, space="PSUM", tag="xgg",
                                   name="xgg", padded_shape=[128, 512])
                    xgo = xps.tile([128, 200], F32, space="PSUM", tag="xgo",
                                   name="xgo", padded_shape=[128, 512])
                    xg_tiles[k] = (xgfi, xgg, xgo)
                    return xg_tiles[k]

                def emit_xg0_mms(k, i):
                    # i-th matmul (of 12) of chunk k: fi kc0-3, g kc0-3, o kc0-3
                    xgfi, xgg, xgo = xg_tiles[k]
                    par = (k // 2) % 2
                    cc = k % 2
                    Ct = [Cf0[par], Cf1[par], Cb0[par], Cb1[par]]
                    kc = i % 4
                    lt = Ct[kc][:, 128 * cc:128 * cc + 128]
                    if i < 4:
                        nc.tensor.matmul(xgfi[:, 0:400], lt, W0t[kc][:, 0:400],
                                         start=(kc == 0), stop=(kc == 3))
                    elif i < 8:
                        nc.tensor.matmul(xgg[:, 0:200], lt, W0t[kc][:, 400:600],
                                         start=(kc == 0), stop=(kc == 3))
                    else:
                        nc.tensor.matmul(xgo[:, 0:200], lt, W0t[kc][:, 600:800],
                                         start=(kc == 0), stop=(kc == 3))

                def emit_xg1_mms(k, i):
                    # i-th matmul (of 24) of chunk k: fi kc0-7, g kc0-7, o kc0-7
                    xgfi, xgg, xgo = xg_tiles[k]
                    par = (k // 4) % 2
                    cc = k % 4
                    Dt = Df[par] + Db[par]
                    kc = i % 8
                    lt = Dt[kc][:, 128 * cc:128 * cc + 128]
                    if i < 8:
                        nc.tensor.matmul(xgfi[:, 0:400], lt,
                                         (W1ft + W1bt)[kc][:, 0:400],
                                         start=(kc == 0), stop=(kc == 7))
                    elif i < 16:
                        nc.tensor.matmul(xgg[:, 0:200], lt,
                                         (W1ft + W1bt)[kc][:, 400:600],
                                         start=(kc == 0), stop=(kc == 7))
                    else:
                        nc.tensor.matmul(xgo[:, 0:200], lt,
                                         (W1ft + W1bt)[kc][:, 600:800],
                                         start=(kc == 0), stop=(kc == 7))

                def emit_gathers(g):
                    par = g % 2
                    nc.gpsimd.indirect_dma_start(
                        out=xf[par][:, 0:E], out_offset=None, in_=emb[:],
                        in_offset=bass.IndirectOffsetOnAxis(ap=tokf_t[:, g:g + 1], axis=0))
                    nc.gpsimd.indirect_dma_start(
                        out=xb[par][:, 0:E], out_offset=None, in_=emb[:],
                        in_offset=bass.IndirectOffsetOnAxis(ap=tokb_t[:, g:g + 1], axis=0))

                def emit_xpose(g):
                    # PE transposes + strided copies into C tiles for group g
                    par = g % 2
                    for (src, C0, C1, dve) in ((xf[par], Cf0[par], Cf1[par], True),
                                               (xb[par], Cb0[par], Cb1[par], False)):
                        half = 0 if dve else 1
                        nc.tensor.transpose(px[:, 0:128], src[:, 0:128], ident128[:])
                        nc.tensor.transpose(px[0:96, 128:256], src[:, 128:224],
                                            ident128[:])
                        d0 = C0[:].rearrange("p (a c b) -> p a c b", c=2, b=16)[:, :, half, :]
                        s0 = px[:, 0:128].rearrange("p (a b) -> p a b", b=16)
                        d1 = C1[0:72].rearrange("p (a c b) -> p a c b", c=2, b=16)[:, :, half, :]
                        s1 = px[0:72, 128:256].rearrange("p (a b) -> p a b", b=16)
                        if dve:
                            nc.vector.tensor_copy(d0, s0)
                            nc.scalar.copy(d1, s1)
                        else:
                            nc.scalar.copy(d0, s0)
                            nc.vector.tensor_copy(d1, s1)

                def emit_step(p, Whht):
                    k = p // CHT
                    r = (p % CHT) * NB
                    sp_ = (p - 1) % NSTG
                    xgfi, xgg, xgo = xg_tiles[k]
                    # recurrent matmuls (accumulate onto x-proj); kc order
                    # 0,2,1,3 matches the h^T mul emission order below
                    KCO = (0, 2, 1, 3)
                    for (t_, w0, w1) in ((xgfi, 0, 400), (xgg, 400, 600),
                                         (xgo, 600, 800)):
                        for ei, kc in enumerate(KCO):
                            nc.tensor.matmul(t_[r:r + NB, 0:w1 - w0],
                                             Ah[kc][:, 32 * sp_:32 * sp_ + 32],
                                             Whht[kc][:, w0:w1],
                                             start=False, stop=(ei == 3),
                                             skip_group_check=True,
                                             tile_position=(0, r))
                    # gate nonlinearities
                    nc.scalar.activation(sigs[:, 0:400], xgfi[r:r + NB, 0:400],
                                         AF.Sigmoid)
                    nc.scalar.activation(S[:, H:2 * H], xgg[r:r + NB, 0:200],
                                         AF.Tanh)
                    nc.scalar.activation(sigs[:, 400:600], xgo[r:r + NB, 0:200],
                                         AF.Sigmoid)
                    # c update
                    nc.vector.tensor_mul(prodt[:], sigs[:, 0:400], S[:])
                    nc.vector.tensor_add(S[:, 0:H], prodt[:, 0:H], prodt[:, H:2 * H])
                    nc.scalar.activation(tcl[:, 0:H], S[:, 0:H], AF.Tanh)

                def emit_step_tail(p):
                    # h^T = so^T * tc^T computed directly in transposed space.
                    # so^T lands in the px bank (stays clear of tclT's bank
                    # marking), is staged to SBUF by Act off the chain; the
                    # DVE muls then read tc^T straight from PSUM.
                    sn = p % NSTG
                    nc.tensor.transpose(px[:, 256:288], sigs[:, 400:528],
                                        ident32[:])
                    nc.tensor.transpose(px[0:96, 288:320], sigs[:, 528:624],
                                        ident32[:])
                    nc.scalar.copy(soT[:, 0:32], px[:, 256:288])
                    nc.scalar.copy(soT[0:72, 32:64], px[0:72, 288:320])
                    nc.tensor.transpose(psTc[:, 0:32], tcl[:, 0:128], ident32[:])
                    nc.tensor.transpose(psTc[0:96, 32:64], tcl[:, 128:224],
                                        ident32[:])
                    nc.vector.tensor_mul(Ah[0][:, 32 * sn:32 * sn + 16],
                                         psTc[:, 0:16], soT[:, 0:16])
                    nc.vector.tensor_mul(Ah[2][:, 32 * sn + 16:32 * sn + 32],
                                         psTc[:, 16:32], soT[:, 16:32])
                    nc.vector.tensor_mul(Ah[1][:, 32 * sn:32 * sn + 16],
                                         psTc[0:72, 32:48], soT[0:72, 32:48])
                    nc.vector.tensor_mul(Ah[3][:, 32 * sn + 16:32 * sn + 32],
                                         psTc[0:72, 48:64], soT[0:72, 48:64])

                def emit_stores(G, hT, T_):
                    # batched h^T stores for 16-step group G
                    c0 = G * NSTG * BL
                    cr0 = (T_ - NSTG * (G + 1)) * BL
                    hTv = hT[:].rearrange("p (t b) -> p t b", b=16)
                    nc.sync.dma_start(out=hTv[0:128, G * NSTG:(G + 1) * NSTG, :],
                                      in_=ahf(0))
                    nc.sync.dma_start(out=hTv[128:200, G * NSTG:(G + 1) * NSTG, :],
                                      in_=ahf(1))
                    rb = T_ // NSTG - 1 - G
                    nc.sync.dma_start(out=hTv[200:328, rb * NSTG:(rb + 1) * NSTG, :],
                                      in_=ahb(2)[:, ::-1, :])
                    nc.sync.dma_start(out=hTv[328:400, rb * NSTG:(rb + 1) * NSTG, :],
                                      in_=ahb(3)[:, ::-1, :])

                def emit_dloads(G):
                    # L1 lhsT loads for 16-step group G (4 chunks)
                    par = G % 2
                    for ri, (r0, r1) in enumerate(RS1):
                        dst = Df[par][ri][:].rearrange(
                            "p (a c b) -> p a c b", c=2, b=16)[:, :, 0, :]
                        src = h0T[r0:r1, G * 256:(G + 1) * 256].rearrange(
                            "p (s b) -> p s b", b=16)
                        nc.sync.dma_start(out=dst, in_=src)
                    rb = NGS - 1 - G
                    for ri, (r0, r1) in enumerate(RS1):
                        dst = Db[par][ri][:].rearrange(
                            "p (a c b) -> p a c b", c=2, b=16)[:, :, 1, :]
                        src = h0T[r0:r1, rb * 256:(rb + 1) * 256].rearrange(
                            "p (s b) -> p s b", b=16)[:, ::-1, :]
                        nc.sync.dma_start(out=dst, in_=src)

                def reset_states():
                    nc.vector.memset(S[:], 0.0)
                    for t_ in Ah:
                        nc.vector.memset(t_[:], 0.0)

                # ================= layer 0 =================
                emit_gathers(0)
                emit_xpose(0)
                alloc_xg(0)
                for i in range(12):
                    emit_xg0_mms(0, i)
                if DEBUG:
                    dstg = sp.tile([128, 400], F32, name="dstg")
                    dstg2 = sp.tile([128, 224], F32, name="dstg2")
                    nc.vector.tensor_copy(dstg2[:], xf[0][:])
                    nc.sync.dma_start(out=DXF[:], in_=dstg2[:])
                    nc.vector.tensor_copy(dstg[:, 0:256], Cf0[0][:])
                    nc.sync.dma_start(out=DCF[:], in_=dstg[:, 0:256])
                    nc.vector.tensor_copy(dstg[0:96, 0:256], Cb1[0][:])
                    nc.sync.dma_start(out=DCB1[:], in_=dstg[0:96, 0:256])
                    nc.vector.tensor_copy(dstg[:], xg_tiles[0][0][:])
                    nc.sync.dma_start(out=DXG[:], in_=dstg[:])
                for G2 in range(NGX):
                    if G2 + 1 < NGX:
                        emit_gathers(G2 + 1)
                    for a in range(XG):
                        p = XG * G2 + a
                        emit_step(p, Whh0t)
                        if G2 + 1 < NGX and a == 2:
                            emit_xpose(G2 + 1)
                        k = p // CHT
                        if k + 1 < NCH:
                            if a % CHT == 0:
                                alloc_xg(k + 1)
                            for q in range(3):
                                emit_xg0_mms(k + 1, (a % CHT) * 3 + q)
                        emit_step_tail(p)
                        if a % CHT == CHT - 1:
                            xg_tiles.pop(k, None)
                        if p % NSTG == NSTG - 1:
                            emit_stores(p // NSTG, h0T, T)
                # ================= layer 1 =================
                reset_states()
                emit_dloads(0)
                alloc_xg(0)
                for i in range(24):
                    emit_xg1_mms(0, i)
                for G in range(NGS):
                    if G + 1 < NGS:
                        emit_dloads(G + 1)
                    for a in range(NSTG):
                        p = NSTG * G + a
                        emit_step(p, Whh1t)
                        k = p // CHT
                        if k + 1 < NCH:
                            if a % CHT == 0:
                                alloc_xg(k + 1)
                            i0 = (a % CHT) * 6
                            for i in range(i0, i0 + 6):
                                emit_xg1_mms(k + 1, i)
                        emit_step_tail(p)
                        if a % CHT == CHT - 1:
                            xg_tiles.pop(k, None)
                    emit_stores(G, h1T, T)

            # ================= U phase =================
            with tc.tile_pool(name="uw", bufs=2) as uw, \
                 tc.tile_pool(name="ups", bufs=2, space="PSUM") as ups:
                for G in range(NSLOT // 512):
                    ut = []
                    for ri, (r0, r1) in enumerate(RSH):
                        t_ = uw.tile([r1 - r0, 512], BF16, tag=f"UL{ri}",
                                     name=f"UL{ri}")
                        nc.sync.dma_start(out=t_[:],
                                          in_=h1T[r0:r1, 512 * G:512 * (G + 1)])
                        ut.append(t_)
                    ustg = uw.tile([128, 3200], BF16, tag="ustg", name="ustg")
                    for cc in range(4):
                        psu0 = ups.tile([128, 400], F32, space="PSUM", tag="psu0",
                                        name="psu0", padded_shape=[128, 512])
                        psu1 = ups.tile([128, 400], F32, space="PSUM", tag="psu1",
                                        name="psu1", padded_shape=[128, 512])
                        for kc in range(4):
                            lt = ut[kc][:, 128 * cc:128 * cc + 128]
                            nc.tensor.matmul(psu0[:], lt, WUt[kc][:, 0:400],
                                             start=(kc == 0), stop=(kc == 3))
                            nc.tensor.matmul(psu1[:], lt, WUt[kc][:, 400:800],
                                             start=(kc == 0), stop=(kc == 3))
                        nc.vector.tensor_copy(ustg[:, 800 * cc:800 * cc + 400],
                                              psu0[:])
                        nc.scalar.copy(ustg[:, 800 * cc + 400:800 * (cc + 1)],
                                       psu1[:])
                    sv = ustg[:].rearrange("p (c j) -> p c j", j=800)
                    d0 = U0d[512 * G:512 * (G + 1), :].rearrange(
                        "(c p) j -> p c j", p=128)
                    d1 = U1d[512 * G:512 * (G + 1), :].rearrange(
                        "(c p) j -> p c j", p=128)
                    nc.sync.dma_start(out=d0, in_=sv[:, :, 0:400])
                    nc.sync.dma_start(out=d1, in_=sv[:, :, 400:800])

            # ================= gather + MLP =================
            with tc.tile_pool(name="fw", bufs=2) as fw, \
                 tc.tile_pool(name="fc", bufs=1) as fc, \
                 tc.tile_pool(name="fps", bufs=2, space="PSUM") as fps:
                ui0 = fc.tile([128, NPT], I32)
                ui1 = fc.tile([128, NPT], I32)
                um0 = fc.tile([128, NPT], F32)
                um1 = fc.tile([128, NPT], F32)
                nc.sync.dma_start(out=ui0[:], in_=uidx0[:])
                nc.sync.dma_start(out=ui1[:], in_=uidx1[:])
                nc.sync.dma_start(out=um0[:], in_=umask0[:])
                nc.sync.dma_start(out=um1[:], in_=umask1[:])
                bwt = fc.tile([128, 2 * H], BF16, name="bwt")
                nc.sync.dma_start(out=bwt[:], in_=bw1m[:])
                hm = [fc.tile([128, 512], BF16, tag=f"hm{i}", name=f"hm{i}")
                      for i in range(2)]
                for t_ in hm:
                    nc.vector.memset(t_[:], 0.0)
                    nc.vector.memset(t_[:, 511:512], 1.0)
                otstg = fc.tile([128, 16], F32, name="otstg")
                for j in range(NPT):
                    par = j % 2
                    g0 = fw.tile([128, 2 * H], BF16, tag="g0", name="g0")
                    g1 = fw.tile([128, 2 * H], BF16, tag="g1", name="g1")
                    nc.gpsimd.indirect_dma_start(
                        out=g0[:], out_offset=None, in_=U0d[:],
                        in_offset=bass.IndirectOffsetOnAxis(ap=ui0[:, j:j + 1], axis=0))
                    nc.gpsimd.indirect_dma_start(
                        out=g1[:], out_offset=None, in_=U1d[:],
                        in_offset=bass.IndirectOffsetOnAxis(ap=ui1[:, j:j + 1], axis=0))
                    g1m = fw.tile([128, 2 * H], BF16, tag="g1m", name="g1m")
                    nc.vector.scalar_tensor_tensor(g1m[:], g1[:], um1[:, j:j + 1],
                                                   bwt[:], ALU.mult, ALU.add)
                    ssum = fw.tile([128, 2 * H], BF16, tag="ssum", name="ssum")
                    nc.vector.scalar_tensor_tensor(ssum[:], g0[:], um0[:, j:j + 1],
                                                   g1m[:], ALU.mult, ALU.add)
                    nc.scalar.activation(hm[par][:, 0:2 * H], ssum[:], AF.Tanh)
                    psl = fps.tile([128, 4], F32, space="PSUM", tag="psl", name="psl")
                    pst = fps.tile([128, 512], BF16, space="PSUM", tag="pst",
                                   name="pst")
                    for i in range(4):
                        nc.tensor.transpose(pst[:, 128 * i:128 * (i + 1)],
                                            hm[par][:, 128 * i:128 * (i + 1)],
                                            ident128[:])
                        hmT = fw.tile([128, 128], BF16, tag=f"hmT{i}", name=f"hmT{i}")
                        if i % 2 == 0:
                            nc.vector.tensor_copy(hmT[:], pst[:, 128 * i:128 * (i + 1)])
                        else:
                            nc.scalar.copy(hmT[:], pst[:, 128 * i:128 * (i + 1)])
                        nc.tensor.matmul(psl[:], hmT[:], W2t[i][:],
                                         start=(i == 0), stop=(i == 3))
                    ex = fw.tile([128, 4], F32, tag="ex", name="ex")
                    nc.scalar.activation(ex[:], psl[:], AF.Exp)
                    if DEBUG and j == 0:
                        dmt = fc.tile([128, 512], F32, name="dmt")
                        nc.vector.tensor_copy(dmt[:, 0:400], g0[:])
                        nc.sync.dma_start(out=DG0[:], in_=dmt[:, 0:400])
                        nc.vector.tensor_copy(dmt[:, 0:400], ssum[:])
                        nc.sync.dma_start(out=DSS[:], in_=dmt[:, 0:400])
                        nc.vector.tensor_copy(dmt[:, 0:512], hm[par][:])
                        nc.sync.dma_start(out=DHM[:], in_=dmt[:, 0:512])
                        nc.vector.tensor_copy(dmt[:, 0:4], ex[:])
                        nc.sync.dma_start(out=DEX[:], in_=dmt[:, 0:4])
                    sm = fw.tile([128, 1], F32, tag="sm", name="sm")
                    nc.vector.reduce_sum(sm[:], ex[:], axis=mybir.AxisListType.X)
                    rc = fw.tile([128, 1], F32, tag="rc", name="rc")
                    nc.vector.reciprocal(rc[:], sm[:])
                    jj = j % 4
                    nc.vector.tensor_scalar_mul(otstg[:, 4 * jj:4 * jj + 4],
                                                ex[:], rc[:, 0:1])
                    if DEBUG and j == 0:
                        dmt2 = fc.tile([128, 8], F32, name="dmt2")
                        nc.vector.tensor_copy(dmt2[:, 0:1], sm[:])
                        nc.vector.tensor_copy(dmt2[:, 1:2], rc[:])
                        nc.vector.tensor_copy(dmt2[:, 2:6], otstg[:, 0:4])
                        nc.sync.dma_start(out=DEX[:], in_=dmt2[:, 2:6])
                    if jj == 3 or j == NPT - 1:
                        nb = jj + 1
                        dst = OUT[128 * (j - jj):128 * (j + 1), :].rearrange(
                            "(c p) j -> p c j", p=128)
                        src = otstg[:, 0:4 * nb].rearrange("p (c j) -> p c j", j=4)
                        nc.sync.dma_start(out=dst, in_=src)
    nc.compile()
    return nc


# ---------------------------------------------------------------------------
# host-side preparation
# ---------------------------------------------------------------------------

def _perm_gates(w):
    """torch gate order (i,f,g,o) -> (f,i,g,o) along axis 0 (4H rows)."""
    Hq = w.shape[0] // 4
    i, f, g, o = (w[0:Hq], w[Hq:2 * Hq], w[2 * Hq:3 * Hq], w[3 * Hq:4 * Hq])
    return np.concatenate([f, i, g, o], axis=0)


def prepare_inputs(inputs, T, n_cores):
    bf = ml_dtypes.bfloat16
    C_ = np.asarray(inputs["confs"]).shape[1]
    emb = np.asarray(inputs["emb"], np.float32)
    tokens = np.asarray(inputs["tokens"])
    confs = np.asarray(inputs["confs"])

    p = {}
    p["emb"] = emb.astype(bf)

    def gp(name):
        return _perm_gates(np.asarray(inputs[name], np.float32))

    Wih0f, Wih0b = gp("Wih0f"), gp("Wih0b")
    b0f, b0b = gp("b0f"), gp("b0b")
    Whh0f, Whh0b = gp("Whh0f"), gp("Whh0b")
    Wih1f, Wih1b = gp("Wih1f"), gp("Wih1b")
    b1f, b1b = gp("b1f"), gp("b1b")
    Whh1f, Whh1b = gp("Whh1f"), gp("Whh1b")
    w1 = np.asarray(inputs["w1"], np.float32)
    bw1 = np.asarray(inputs["bw1"], np.float32)
    w2 = np.asarray(inputs["w2"], np.float32)
    bw2 = np.asarray(inputs["bw2"], np.float32)

    # W0s: x-proj chunks (128, 96, 128, 96); bias at row 95 of 96-chunks
    w0 = np.zeros((448, G4), np.float32)
    w0[0:128] = Wih0f.T[0:128]
    w0[128:200] = Wih0f.T[128:200]
    w0[223] = b0f
    w0[224:352] = Wih0b.T[0:128]
    w0[352:424] = Wih0b.T[128:200]
    w0[447] = b0b
    p["W0s"] = w0.astype(bf)

    def rec_stream(wf, wb):
        o = np.zeros((400, G4), np.float32)
        o[0:128] = wf.T[0:128]
        o[128:200] = wf.T[128:200]
        o[200:328] = wb.T[0:128]
        o[328:400] = wb.T[128:200]
        return o.astype(bf)

    p["Whh0s"] = rec_stream(Whh0f, Whh0b)
    p["Whh1s"] = rec_stream(Whh1f, Whh1b)
    p["W1sf"] = np.concatenate([Wih1f.T, b1f[None, :]], 0).astype(bf)
    p["W1sb"] = np.concatenate([Wih1b.T, b1b[None, :]], 0).astype(bf)

    # WUs: K = h1 feature, N = [U0 cols | U1 cols]
    w1T = w1.T  # [800, 400]
    wu = np.zeros((400, G4), np.float32)
    wu[:, 0:400] = w1T[0:400]
    wu[:, 400:800] = w1T[400:800]
    p["WUs"] = wu.astype(bf)
    p["bw1m"] = np.tile(bw1[None, :], (128, 1)).astype(bf)
    w2p = np.zeros((512, 4), np.float32)
    w2p[0:2 * H] = w2.T
    w2p[511] = bw2
    p["W2s"] = w2p.astype(bf)

    NGX = T // XG
    NP = BL * C_
    NPT = (NP + 127) // 128

    in_maps = []
    for c in range(n_cores):
        m = dict(p)
        bs = tokens[c * BL:(c + 1) * BL, 0:T]          # [BL, T]
        tf = np.zeros((128, NGX), np.int32)
        tb = np.zeros((128, NGX), np.int32)
        for g in range(NGX):
            for a in range(XG):
                tf[a * BL:(a + 1) * BL, g] = bs[:, XG * g + a]
                tb[a * BL:(a + 1) * BL, g] = bs[:, T - 1 - (XG * g + a)]
        m["tokf"] = tf
        m["tokb"] = tb
        cf = confs[c * BL:(c + 1) * BL]                 # [BL, C, 2]
        t0 = cf[:, :, 0].reshape(-1)                    # row-major (b, ci)
        t1 = cf[:, :, 1].reshape(-1)
        bidx = np.repeat(np.arange(BL), C_)
        ui0 = np.clip(t0, 0, T - 1) * BL + bidx
        ui1 = np.clip(t1, 0, T - 1) * BL + bidx
        um0 = (t0 >= 0).astype(np.float32)
        um1 = (t1 >= 0).astype(np.float32)

        def tile128(a, dt):
            o = np.zeros((NPT * 128,), dt)
            o[:a.shape[0]] = a
            return o.reshape(NPT, 128).T.copy()
        m["uidx0"] = tile128(ui0.astype(np.int32), np.int32)
        m["uidx1"] = tile128(ui1.astype(np.int32), np.int32)
        m["umask0"] = tile128(um0, np.float32)
        m["umask1"] = tile128(um1, np.float32)
        in_maps.append(m)
    return in_maps


_CACHE = {}


def _get_prog(T, n_cores, NPT):
    key = (T, n_cores, NPT)
    if key not in _CACHE:
        _CACHE[key] = build(T, n_cores, NPT)
    return _CACHE[key]


def kernel(**inputs):
    T = inputs["tokens"].shape[1]
    C_ = inputs["confs"].shape[1]
    n_cores = NCORE
    NP = BL * C_
    NPT = (NP + 127) // 128
    nc = _get_prog(T, n_cores, NPT)
    in_maps = prepare_inputs(inputs, T, n_cores)
    res = run_bass_kernel_spmd(nc, in_maps, list(range(n_cores)))
    outs = []
    for c in range(n_cores):
        o = res.results[c]["OUT"][:NP]          # [BL*C, 4] rows (b, ci)
        outs.append(o)
    return np.concatenate(outs, axis=0).astype(np.float32)


# revision 55
# speedup vs baseline: 1.1698x; 1.0793x over previous
"""Trainium2 Bass kernel for nn_BiLSTMNet (2-layer BiLSTM + pair-gather MLP).

v2 design (latency-focused):
- Data-parallel across 8 cores (16 sentences each), fwd+bwd fused per step
  (32 rows) via block-diagonal K in the recurrent matmul.
- h^T lhsT tiles live in 16-step SBUF *history* buffers; DRAM h stores are
  batched to 4 DMAs per 16 steps (vs 8/step in v1) - kills the HWDGE
  descriptor-generation bottleneck (625ns per DMA instruction, single queue).
- bf16 elementwise chain (sigmoids/tanh outputs, cell state, products) for
  DVE 2x mode; fp32 only in PSUM matmul accumulators.
- x embedding gathers via Pool/SWDGE; x^T staging via PE transposes (no
  DMA-transposes).
- bwd-direction stores / loads use reversed-block access patterns so h0T/h1T
  hold time-aligned [hf(t); hb(t)] and every DMA is a single descriptor run.
- U = h1 @ w1^T phase batched (4-chunk groups); MLP pair-gather via SWDGE.
"""
import sys
sys.path.insert(0, "/opt/trn_rl_repo")
import numpy as np
import ml_dtypes

import concourse.bass as bass
import concourse.tile as tile
from concourse import mybir, bacc
from concourse.bass_utils import run_bass_kernel_spmd
from concourse.masks import make_identity

BF16 = mybir.dt.bfloat16
F32 = mybir.dt.float32
I32 = mybir.dt.int32
AF = mybir.ActivationFunctionType
ALU = mybir.AluOpType

V, E, H, B, C = 32000, 200, 200, 128, 256
T_FULL = 512
BL = 16            # sentences per core
NB = 32            # rows per step (16 fwd + 16 bwd)
NCORE = 8
CHT = 4            # steps per PSUM chunk (4*32 = 128 slots)
XG = 8             # steps per x-gather group (2 chunks)
NSTG = 16          # steps per store / D-load group
G4 = 800           # 4*H gate width


def build(T, n_cores, NPT):
    NSLOT = T * BL
    NCH = T // CHT
    NGX = T // XG
    NGS = T // NSTG

    nc = bacc.Bacc("TRN2", target_bir_lowering=False, debug=False,
                   enable_asserts=True, num_devices=n_cores)

    def din(name, shape, dt):
        return nc.dram_tensor(name, shape, dt, kind="ExternalInput").ap()

    def dout(name, shape, dt):
        return nc.dram_tensor(name, shape, dt, kind="ExternalOutput").ap()

    emb = din("emb", [V, E], BF16)
    W0s = din("W0s", [448, G4], BF16)      # x-proj chunks (128,96,128,96), bias@row95
    Whh0s = din("Whh0s", [400, G4], BF16)  # rec chunks (128,72,128,72)
    W1sf = din("W1sf", [401, G4], BF16)    # L1 x-proj fwd (128,128,128,17)
    W1sb = din("W1sb", [401, G4], BF16)
    Whh1s = din("Whh1s", [400, G4], BF16)
    WUs = din("WUs", [400, G4], BF16)      # U chunks (128,72,128,72)
    W2s = din("W2s", [512, 4], BF16)
    tokf = din("tokf", [128, NGX], I32)
    tokb = din("tokb", [128, NGX], I32)
    uidx0 = din("uidx0", [128, NPT], I32)
    uidx1 = din("uidx1", [128, NPT], I32)
    umask0 = din("umask0", [128, NPT], F32)
    umask1 = din("umask1", [128, NPT], F32)
    bw1m = din("bw1m", [128, 2 * H], BF16)

    OUT = dout("OUT", [NPT * 128, 4], F32)
    import os
    DEBUG = os.environ.get("KDEBUG", "") == "1"
    if DEBUG:
        DXF = dout("DXF", [128, 224], F32)
        DCF = dout("DCF", [128, 256], F32)
        DCB1 = dout("DCB1", [96, 256], F32)
        DXG = dout("DXG", [128, 400], F32)
        DHH = dout("DHH", [32, 224], F32)
        DA0 = dout("DA0", [128, 512], F32)
        DSG = dout("DSG", [32, 600], F32)
        DS = dout("DS", [32, 400], F32)
        DG0 = dout("DG0", [128, 400], F32)
        DSS = dout("DSS", [128, 400], F32)
        DHM = dout("DHM", [128, 512], F32)
        DEX = dout("DEX", [128, 4], F32)

    # internal DRAM: time-aligned h^T ([hf(t);hb(t)] at col block t)
    h0T = nc.dram_tensor("h0T", [401, NSLOT], BF16).ap()   # row 400 = ones
    h1T = nc.dram_tensor("h1T", [400, NSLOT], BF16).ap()
    U0d = nc.dram_tensor("U0d", [NSLOT, 2 * H], BF16).ap()
    U1d = nc.dram_tensor("U1d", [NSLOT, 2 * H], BF16).ap()

    RS0 = [(0, 128), (128, 224), (224, 352), (352, 448)]   # W0s chunks
    RSH = [(0, 128), (128, 200), (200, 328), (328, 400)]   # Whh/WU chunks
    RS1 = [(0, 128), (128, 256), (256, 384), (384, 401)]   # L1 x chunks

    with tile.TileContext(nc) as tc:
        with tc.tile_pool(name="const", bufs=1) as cp, \
             tc.tile_pool(name="state", bufs=1) as sp:

            def load_chunks(src, rowsets, ncols, pfx):
                tiles = []
                for i, (r0, r1) in enumerate(rowsets):
                    t_ = cp.tile([r1 - r0, ncols], BF16, tag=f"{pfx}{i}",
                                 name=f"{pfx}{i}")
                    nc.sync.dma_start(out=t_[:], in_=src[r0:r1, :])
                    tiles.append(t_)
                return tiles

            W0t = load_chunks(W0s, RS0, G4, "W0")
            Whh0t = load_chunks(Whh0s, RSH, G4, "Wh0")
            W1ft = load_chunks(W1sf, RS1, G4, "W1f")
            W1bt = load_chunks(W1sb, RS1, G4, "W1b")
            Whh1t = load_chunks(Whh1s, RSH, G4, "Wh1")
            WUt = load_chunks(WUs, RSH, G4, "WU")
            W2t = load_chunks(W2s, [(0, 128), (128, 256), (256, 384), (384, 512)],
                              4, "W2")

            tokf_t = cp.tile([128, NGX], I32)
            tokb_t = cp.tile([128, NGX], I32)
            nc.sync.dma_start(out=tokf_t[:], in_=tokf[:])
            nc.sync.dma_start(out=tokb_t[:], in_=tokb[:])

            ones_row = cp.tile([1, NSLOT], BF16)
            nc.vector.memset(ones_row[:], 1.0)
            nc.sync.dma_start(out=h0T[400:401, :], in_=ones_row[:])

            ident32 = sp.tile([32, 32], BF16, name="ident32")
            ident128 = sp.tile([128, 128], BF16, name="ident128")
            make_identity(nc, ident32[:])
            make_identity(nc, ident128[:])

            # ---- persistent state
            # h^T history buffers (16 step slots x 32 cols; f cols 0:16, b 16:32)
            Ah = [sp.tile([128, 32 * NSTG], BF16, tag="A0h", name="A0h"),
                  sp.tile([72, 32 * NSTG], BF16, tag="A1h", name="A1h"),
                  sp.tile([128, 32 * NSTG], BF16, tag="A2h", name="A2h"),
                  sp.tile([72, 32 * NSTG], BF16, tag="A3h", name="A3h")]
            S = sp.tile([NB, 2 * H], BF16, name="S")        # c | tanh(g)
            sigs = sp.tile([NB, 624], BF16, name="sigs")    # s(f),s(i) | s(o) | pad
            tcl = sp.tile([NB, 224], BF16, name="tcl")      # tanh(c) | pad
            prodt = sp.tile([NB, 2 * H], BF16, name="prodt")
            soT = sp.tile([128, 64], BF16, name="soT")      # s(o)^T staging
            tcT = sp.tile([128, 64], BF16, name="tcT")      # tanh(c)^T staging
            # x gather tiles + x^T lhsT (2-chunk groups, parity)
            xf = [sp.tile([128, 224], BF16, tag=f"xf{i}", name=f"xf{i}") for i in range(2)]
            xb = [sp.tile([128, 224], BF16, tag=f"xb{i}", name=f"xb{i}") for i in range(2)]
            Cf0 = [sp.tile([128, 256], BF16, tag=f"Cf0{i}", name=f"Cf0{i}") for i in range(2)]
            Cf1 = [sp.tile([96, 256], BF16, tag=f"Cf1{i}", name=f"Cf1{i}") for i in range(2)]
            Cb0 = [sp.tile([128, 256], BF16, tag=f"Cb0{i}", name=f"Cb0{i}") for i in range(2)]
            Cb1 = [sp.tile([96, 256], BF16, tag=f"Cb1{i}", name=f"Cb1{i}") for i in range(2)]
            # L1 lhsT tiles (4-chunk groups, parity); b-halves/f-halves stay 0
            Df = [[sp.tile([r1 - r0, 512], BF16, tag=f"Df{i}{j}", name=f"Df{i}{j}")
                   for i, (r0, r1) in enumerate(RS1)] for j in range(2)]
            Db = [[sp.tile([r1 - r0, 512], BF16, tag=f"Db{i}{j}", name=f"Db{i}{j}")
                   for i, (r0, r1) in enumerate(RS1)] for j in range(2)]

            for t_ in [S, sigs, tcl, prodt, soT, tcT] + Ah + xf + xb:
                nc.vector.memset(t_[:], 0.0)
            for j in range(2):
                for t_ in [Cf0[j], Cf1[j], Cb0[j], Cb1[j]] + Df[j] + Db[j]:
                    nc.vector.memset(t_[:], 0.0)
                # bias rows: 1.0 only in the direction's own column halves
                # (engine ops can't start at partition 95 -> use SBUF-SBUF DMA)
                ov = ones_row[:, 0:128].rearrange("p (a b) -> p a b", b=16)
                vf = Cf1[j][95:96, :].rearrange("p (a c b) -> p a c b", c=2, b=16)
                nc.sync.dma_start(out=vf[:, :, 0, :], in_=ov)
                vb = Cb1[j][95:96, :].rearrange("p (a c b) -> p a c b", c=2, b=16)
                nc.sync.dma_start(out=vb[:, :, 1, :], in_=ov)

            def ahf(i):
                # f-half view [p, s, 16] of history i
                return Ah[i][:].rearrange("p (s c b) -> p s c b", c=2, b=16)[:, :, 0, :]

            def ahb(i):
                return Ah[i][:].rearrange("p (s c b) -> p s c b", c=2, b=16)[:, :, 1, :]

            with tc.tile_pool(name="xps", bufs=2, space="PSUM") as xps, \
                 tc.tile_pool(name="tps", bufs=1, space="PSUM") as tps:

                # persistent PSUM staging tiles. Zero-region (bank) granularity
                # matters: a transpose's start=True marks its whole bank, so
                # readers of OTHER regions in that bank get serialized after
                # it. tc^T gets its own bank; so^T shares the x-stage bank
                # (written only once per 8 steps).
                psTc = tps.tile([128, 64], BF16, tag="psTc", name="psTc")
                px = tps.tile([128, 320], BF16, tag="px", name="px")

                xg_tiles = {}

                def alloc_xg(k):
                    # fi / g / o in separate banks: each gate group start=True
                    # marks its whole 2KB zero region, so groups can't share
                    xgfi = xps.tile([128, 400], F32, space="PSUM", tag="xgfi",
                                    name="xgfi", padded_shape=[128, 512])
                    xgg = xps.tile([128, 200], F32, space="PSUM", tag="xgg",
                                   name="xgg", padded_shape=[128, 512])
                    xgo = xps.tile([128, 200], F32, space="PSUM", tag="xgo",
                                   name="xgo", padded_shape=[128, 512])
                    xg_tiles[k] = (xgfi, xgg, xgo)
                    return xg_tiles[k]

                def emit_xg0_mms(k, i):
                    # i-th matmul (of 12) of chunk k: fi kc0-3, g kc0-3, o kc0-3
                    xgfi, xgg, xgo = xg_tiles[k]
                    par = (k // 2) % 2
                    cc = k % 2
                    Ct = [Cf0[par], Cf1[par], Cb0[par], Cb1[par]]
                    kc = i % 4
                    lt = Ct[kc][:, 128 * cc:128 * cc + 128]
                    if i < 4:
                        nc.tensor.matmul(xgfi[:, 0:400], lt, W0t[kc][:, 0:400],
                                         start=(kc == 0), stop=(kc == 3))
                    elif i < 8:
                        nc.tensor.matmul(xgg[:, 0:200], lt, W0t[kc][:, 400:600],
                                         start=(kc == 0), stop=(kc == 3))
                    else:
                        nc.tensor.matmul(xgo[:, 0:200], lt, W0t[kc][:, 600:800],
                                         start=(kc == 0), stop=(kc == 3))

                def emit_xg1_mms(k, i):
                    # i-th matmul (of 24) of chunk k: fi kc0-7, g kc0-7, o kc0-7
                    xgfi, xgg, xgo = xg_tiles[k]
                    par = (k // 4) % 2
                    cc = k % 4
                    Dt = Df[par] + Db[par]
                    kc = i % 8
                    lt = Dt[kc][:, 128 * cc:128 * cc + 128]
                    if i < 8:
                        nc.tensor.matmul(xgfi[:, 0:400], lt,
                                         (W1ft + W1bt)[kc][:, 0:400],
                                         start=(kc == 0), stop=(kc == 7))
                    elif i < 16:
                        nc.tensor.matmul(xgg[:, 0:200], lt,
                                         (W1ft + W1bt)[kc][:, 400:600],
                                         start=(kc == 0), stop=(kc == 7))
                    else:
                        nc.tensor.matmul(xgo[:, 0:200], lt,
                                         (W1ft + W1bt)[kc][:, 600:800],
                                         start=(kc == 0), stop=(kc == 7))

                def emit_gathers(g):
                    par = g % 2
                    nc.gpsimd.indirect_dma_start(
                        out=xf[par][:, 0:E], out_offset=None, in_=emb[:],
                        in_offset=bass.IndirectOffsetOnAxis(ap=tokf_t[:, g:g + 1], axis=0))
                    nc.gpsimd.indirect_dma_start(
                        out=xb[par][:, 0:E], out_offset=None, in_=emb[:],
                        in_offset=bass.IndirectOffsetOnAxis(ap=tokb_t[:, g:g + 1], axis=0))

                def emit_xpose(g):
                    # PE transposes + strided copies into C tiles for group g
                    par = g % 2
                    for (src, C0, C1, dve) in ((xf[par], Cf0[par], Cf1[par], True),
                                               (xb[par], Cb0[par], Cb1[par], False)):
                        half = 0 if dve else 1
                        nc.tensor.transpose(px[:, 0:128], src[:, 0:128], ident128[:])
                        nc.tensor.transpose(px[0:96, 128:256], src[:, 128:224],
                                            ident128[:])
                        d0 = C0[:].rearrange("p (a c b) -> p a c b", c=2, b=16)[:, :, half, :]
                        s0 = px[:, 0:128].rearrange("p (a b) -> p a b", b=16)
                        d1 = C1[0:72].rearrange("p (a c b) -> p a c b", c=2, b=16)[:, :, half, :]
                        s1 = px[0:72, 128:256].rearrange("p (a b) -> p a b", b=16)
                        if dve:
                            nc.vector.tensor_copy(d0, s0)
                            nc.scalar.copy(d1, s1)
                        else:
                            nc.scalar.copy(d0, s0)
                            nc.vector.tensor_copy(d1, s1)

                def emit_step(p, Whht):
                    k = p // CHT
                    r = (p % CHT) * NB
                    sp_ = (p - 1) % NSTG
                    xgfi, xgg, xgo = xg_tiles[k]
                    # recurrent matmuls (accumulate onto x-proj); kc order
                    # 0,2,1,3 matches the h^T mul emission order below
                    KCO = (0, 2, 1, 3)
                    for (t_, w0, w1) in ((xgfi, 0, 400), (xgg, 400, 600),
                                         (xgo, 600, 800)):
                        for ei, kc in enumerate(KCO):
                            nc.tensor.matmul(t_[r:r + NB, 0:w1 - w0],
                                             Ah[kc][:, 32 * sp_:32 * sp_ + 32],
                                             Whht[kc][:, w0:w1],
                                             start=False, stop=(ei == 3),
                                             skip_group_check=True,
                                             tile_position=(0, r))
                    # gate nonlinearities
                    nc.scalar.activation(sigs[:, 0:400], xgfi[r:r + NB, 0:400],
                                         AF.Sigmoid)
                    nc.scalar.activation(S[:, H:2 * H], xgg[r:r + NB, 0:200],
                                         AF.Tanh)
                    nc.scalar.activation(sigs[:, 400:600], xgo[r:r + NB, 0:200],
                                         AF.Sigmoid)
                    # c update
                    nc.vector.tensor_mul(prodt[:], sigs[:, 0:400], S[:])
                    nc.vector.tensor_add(S[:, 0:H], prodt[:, 0:H], prodt[:, H:2 * H])
                    nc.scalar.activation(tcl[:, 0:H], S[:, 0:H], AF.Tanh)

                def emit_step_tail(p):
                    # h^T = so^T * tc^T computed directly in transposed space.
                    # so^T lands in the px bank (stays clear of tclT's bank
                    # marking), is staged to SBUF by Act off the chain; the
                    # DVE muls then read tc^T straight from PSUM.
                    sn = p % NSTG
                    nc.tensor.transpose(px[:, 256:288], sigs[:, 400:528],
                                        ident32[:])
                    nc.tensor.transpose(px[0:96, 288:320], sigs[:, 528:624],
                                        ident32[:])
                    nc.vector.tensor_copy(soT[:, 0:32], px[:, 256:288])
                    nc.vector.tensor_copy(soT[0:72, 32:64], px[0:72, 288:320])
                    nc.tensor.transpose(psTc[:, 0:32], tcl[:, 0:128], ident32[:])
                    nc.tensor.transpose(psTc[0:96, 32:64], tcl[:, 128:224],
                                        ident32[:])
                    nc.vector.tensor_copy(tcT[:, 0:32], psTc[:, 0:32])
                    nc.vector.tensor_copy(tcT[0:72, 32:64], psTc[0:72, 32:64])
                    nc.vector.tensor_mul(Ah[0][:, 32 * sn:32 * sn + 16],
                                         tcT[:, 0:16], soT[:, 0:16])
                    nc.vector.tensor_mul(Ah[2][:, 32 * sn + 16:32 * sn + 32],
                                         tcT[:, 16:32], soT[:, 16:32])
                    nc.vector.tensor_mul(Ah[1][:, 32 * sn:32 * sn + 16],
                                         tcT[0:72, 32:48], soT[0:72, 32:48])
                    nc.vector.tensor_mul(Ah[3][:, 32 * sn + 16:32 * sn + 32],
                                         tcT[0:72, 48:64], soT[0:72, 48:64])

                def emit_stores(G, hT, T_):
                    # batched h^T stores for 16-step group G
                    c0 = G * NSTG * BL
                    cr0 = (T_ - NSTG * (G + 1)) * BL
                    hTv = hT[:].rearrange("p (t b) -> p t b", b=16)
                    nc.sync.dma_start(out=hTv[0:128, G * NSTG:(G + 1) * NSTG, :],
                                      in_=ahf(0))
                    nc.sync.dma_start(out=hTv[128:200, G * NSTG:(G + 1) * NSTG, :],
                                      in_=ahf(1))
                    rb = T_ // NSTG - 1 - G
                    nc.sync.dma_start(out=hTv[200:328, rb * NSTG:(rb + 1) * NSTG, :],
                                      in_=ahb(2)[:, ::-1, :])
                    nc.sync.dma_start(out=hTv[328:400, rb * NSTG:(rb + 1) * NSTG, :],
                                      in_=ahb(3)[:, ::-1, :])

                def emit_dloads(G):
                    # L1 lhsT loads for 16-step group G (4 chunks)
                    par = G % 2
                    for ri, (r0, r1) in enumerate(RS1):
                        dst = Df[par][ri][:].rearrange(
                            "p (a c b) -> p a c b", c=2, b=16)[:, :, 0, :]
                        src = h0T[r0:r1, G * 256:(G + 1) * 256].rearrange(
                            "p (s b) -> p s b", b=16)
                        nc.sync.dma_start(out=dst, in_=src)
                    rb = NGS - 1 - G
                    for ri, (r0, r1) in enumerate(RS1):
                        dst = Db[par][ri][:].rearrange(
                            "p (a c b) -> p a c b", c=2, b=16)[:, :, 1, :]
                        src = h0T[r0:r1, rb * 256:(rb + 1) * 256].rearrange(
                            "p (s b) -> p s b", b=16)[:, ::-1, :]
                        nc.sync.dma_start(out=dst, in_=src)

                def reset_states():
                    nc.vector.memset(S[:], 0.0)
                    for t_ in Ah:
                        nc.vector.memset(t_[:], 0.0)

                # ================= layer 0 =================
                emit_gathers(0)
                emit_xpose(0)
                alloc_xg(0)
                for i in range(12):
                    emit_xg0_mms(0, i)
                if DEBUG:
                    dstg = sp.tile([128, 400], F32, name="dstg")
                    dstg2 = sp.tile([128, 224], F32, name="dstg2")
                    nc.vector.tensor_copy(dstg2[:], xf[0][:])
                    nc.sync.dma_start(out=DXF[:], in_=dstg2[:])
                    nc.vector.tensor_copy(dstg[:, 0:256], Cf0[0][:])
                    nc.sync.dma_start(out=DCF[:], in_=dstg[:, 0:256])
                    nc.vector.tensor_copy(dstg[0:96, 0:256], Cb1[0][:])
                    nc.sync.dma_start(out=DCB1[:], in_=dstg[0:96, 0:256])
                    nc.vector.tensor_copy(dstg[:], xg_tiles[0][0][:])
                    nc.sync.dma_start(out=DXG[:], in_=dstg[:])
                for G2 in range(NGX):
                    if G2 + 1 < NGX:
                        emit_gathers(G2 + 1)
                    for a in range(XG):
                        p = XG * G2 + a
                        emit_step(p, Whh0t)
                        if G2 + 1 < NGX and a == 2:
                            emit_xpose(G2 + 1)
                        k = p // CHT
                        if k + 1 < NCH:
                            if a % CHT == 0:
                                alloc_xg(k + 1)
                            for q in range(3):
                                emit_xg0_mms(k + 1, (a % CHT) * 3 + q)
                        emit_step_tail(p)
                        if a % CHT == CHT - 1:
                            xg_tiles.pop(k, None)
                        if p % NSTG == NSTG - 1:
                            emit_stores(p // NSTG, h0T, T)
                # ================= layer 1 =================
                reset_states()
                emit_dloads(0)
                alloc_xg(0)
                for i in range(24):
                    emit_xg1_mms(0, i)
                for G in range(NGS):
                    if G + 1 < NGS:
                        emit_dloads(G + 1)
                    for a in range(NSTG):
                        p = NSTG * G + a
                        emit_step(p, Whh1t)
                        k = p // CHT
                        if k + 1 < NCH:
                            if a % CHT == 0:
                                alloc_xg(k + 1)
                            i0 = (a % CHT) * 6
                            for i in range(i0, i0 + 6):
                                emit_xg1_mms(k + 1, i)
                        emit_step_tail(p)
                        if a % CHT == CHT - 1:
                            xg_tiles.pop(k, None)
                    emit_stores(G, h1T, T)

            # ================= U phase =================
            with tc.tile_pool(name="uw", bufs=2) as uw, \
                 tc.tile_pool(name="ups", bufs=2, space="PSUM") as ups:
                for G in range(NSLOT // 512):
                    ut = []
                    for ri, (r0, r1) in enumerate(RSH):
                        t_ = uw.tile([r1 - r0, 512], BF16, tag=f"UL{ri}",
                                     name=f"UL{ri}")
                        nc.sync.dma_start(out=t_[:],
                                          in_=h1T[r0:r1, 512 * G:512 * (G + 1)])
                        ut.append(t_)
                    ustg = uw.tile([128, 3200], BF16, tag="ustg", name="ustg")
                    for cc in range(4):
                        psu0 = ups.tile([128, 400], F32, space="PSUM", tag="psu0",
                                        name="psu0", padded_shape=[128, 512])
                        psu1 = ups.tile([128, 400], F32, space="PSUM", tag="psu1",
                                        name="psu1", padded_shape=[128, 512])
                        for kc in range(4):
                            lt = ut[kc][:, 128 * cc:128 * cc + 128]
                            nc.tensor.matmul(psu0[:], lt, WUt[kc][:, 0:400],
                                             start=(kc == 0), stop=(kc == 3))
                            nc.tensor.matmul(psu1[:], lt, WUt[kc][:, 400:800],
                                             start=(kc == 0), stop=(kc == 3))
                        nc.vector.tensor_copy(ustg[:, 800 * cc:800 * cc + 400],
                                              psu0[:])
                        nc.scalar.copy(ustg[:, 800 * cc + 400:800 * (cc + 1)],
                                       psu1[:])
                    sv = ustg[:].rearrange("p (c j) -> p c j", j=800)
                    d0 = U0d[512 * G:512 * (G + 1), :].rearrange(
                        "(c p) j -> p c j", p=128)
                    d1 = U1d[512 * G:512 * (G + 1), :].rearrange(
                        "(c p) j -> p c j", p=128)
                    nc.sync.dma_start(out=d0, in_=sv[:, :, 0:400])
                    nc.sync.dma_start(out=d1, in_=sv[:, :, 400:800])

            # ================= gather + MLP =================
            with tc.tile_pool(name="fw", bufs=2) as fw, \
                 tc.tile_pool(name="fc", bufs=1) as fc, \
                 tc.tile_pool(name="fps", bufs=2, space="PSUM") as fps:
                ui0 = fc.tile([128, NPT], I32)
                ui1 = fc.tile([128, NPT], I32)
                um0 = fc.tile([128, NPT], F32)
                um1 = fc.tile([128, NPT], F32)
                nc.sync.dma_start(out=ui0[:], in_=uidx0[:])
                nc.sync.dma_start(out=ui1[:], in_=uidx1[:])
                nc.sync.dma_start(out=um0[:], in_=umask0[:])
                nc.sync.dma_start(out=um1[:], in_=umask1[:])
                bwt = fc.tile([128, 2 * H], BF16, name="bwt")
                nc.sync.dma_start(out=bwt[:], in_=bw1m[:])
                hm = [fc.tile([128, 512], BF16, tag=f"hm{i}", name=f"hm{i}")
                      for i in range(2)]
                for t_ in hm:
                    nc.vector.memset(t_[:], 0.0)
                    nc.vector.memset(t_[:, 511:512], 1.0)
                otstg = fc.tile([128, 16], F32, name="otstg")
                for j in range(NPT):
                    par = j % 2
                    g0 = fw.tile([128, 2 * H], BF16, tag="g0", name="g0")
                    g1 = fw.tile([128, 2 * H], BF16, tag="g1", name="g1")
                    nc.gpsimd.indirect_dma_start(
                        out=g0[:], out_offset=None, in_=U0d[:],
                        in_offset=bass.IndirectOffsetOnAxis(ap=ui0[:, j:j + 1], axis=0))
                    nc.gpsimd.indirect_dma_start(
                        out=g1[:], out_offset=None, in_=U1d[:],
                        in_offset=bass.IndirectOffsetOnAxis(ap=ui1[:, j:j + 1], axis=0))
                    g1m = fw.tile([128, 2 * H], BF16, tag="g1m", name="g1m")
                    nc.vector.scalar_tensor_tensor(g1m[:], g1[:], um1[:, j:j + 1],
                                                   bwt[:], ALU.mult, ALU.add)
                    ssum = fw.tile([128, 2 * H], BF16, tag="ssum", name="ssum")
                    nc.vector.scalar_tensor_tensor(ssum[:], g0[:], um0[:, j:j + 1],
                                                   g1m[:], ALU.mult, ALU.add)
                    nc.scalar.activation(hm[par][:, 0:2 * H], ssum[:], AF.Tanh)
                    psl = fps.tile([128, 4], F32, space="PSUM", tag="psl", name="psl")
                    pst = fps.tile([128, 512], BF16, space="PSUM", tag="pst",
                                   name="pst")
                    for i in range(4):
                        nc.tensor.transpose(pst[:, 128 * i:128 * (i + 1)],
                                            hm[par][:, 128 * i:128 * (i + 1)],
                                            ident128[:])
                        hmT = fw.tile([128, 128], BF16, tag=f"hmT{i}", name=f"hmT{i}")
                        if i % 2 == 0:
                            nc.vector.tensor_copy(hmT[:], pst[:, 128 * i:128 * (i + 1)])
                        else:
                            nc.scalar.copy(hmT[:], pst[:, 128 * i:128 * (i + 1)])
                        nc.tensor.matmul(psl[:], hmT[:], W2t[i][:],
                                         start=(i == 0), stop=(i == 3))
                    ex = fw.tile([128, 4], F32, tag="ex", name="ex")
                    nc.scalar.activation(ex[:], psl[:], AF.Exp)
                    if DEBUG and j == 0:
                        dmt = fc.tile([128, 512], F32, name="dmt")
                        nc.vector.tensor_copy(dmt[:, 0:400], g0[:])
                        nc.sync.dma_start(out=DG0[:], in_=dmt[:, 0:400])
                        nc.vector.tensor_copy(dmt[:, 0:400], ssum[:])
                        nc.sync.dma_start(out=DSS[:], in_=dmt[:, 0:400])
                        nc.vector.tensor_copy(dmt[:, 0:512], hm[par][:])
                        nc.sync.dma_start(out=DHM[:], in_=dmt[:, 0:512])
                        nc.vector.tensor_copy(dmt[:, 0:4], ex[:])
                        nc.sync.dma_start(out=DEX[:], in_=dmt[:, 0:4])
                    sm = fw.tile([128, 1], F32, tag="sm", name="sm")
                    nc.vector.reduce_sum(sm[:], ex[:], axis=mybir.AxisListType.X)
                    rc = fw.tile([128, 1], F32, tag="rc", name="rc")
                    nc.vector.reciprocal(rc[:], sm[:])
                    jj = j % 4
                    nc.vector.tensor_scalar_mul(otstg[:, 4 * jj:4 * jj + 4],
                                                ex[:], rc[:, 0:1])
                    if DEBUG and j == 0:
                        dmt2 = fc.tile([128, 8], F32, name="dmt2")
                        nc.vector.tensor_copy(dmt2[:, 0:1], sm[:])
                        nc.vector.tensor_copy(dmt2[:, 1:2], rc[:])
                        nc.vector.tensor_copy(dmt2[:, 2:6], otstg[:, 0:4])
                        nc.sync.dma_start(out=DEX[:], in_=dmt2[:, 2:6])
                    if jj == 3 or j == NPT - 1:
                        nb = jj + 1
                        dst = OUT[128 * (j - jj):128 * (j + 1), :].rearrange(
                            "(c p) j -> p c j", p=128)
                        src = otstg[:, 0:4 * nb].rearrange("p (c j) -> p c j", j=4)
                        nc.sync.dma_start(out=dst, in_=src)
    nc.compile()
    return nc


# ---------------------------------------------------------------------------
# host-side preparation
# ---------------------------------------------------------------------------

def _perm_gates(w):
    """torch gate order (i,f,g,o) -> (f,i,g,o) along axis 0 (4H rows)."""
    Hq = w.shape[0] // 4
    i, f, g, o = (w[0:Hq], w[Hq:2 * Hq], w[2 * Hq:3 * Hq], w[3 * Hq:4 * Hq])
    return np.concatenate([f, i, g, o], axis=0)


def prepare_inputs(inputs, T, n_cores):
    bf = ml_dtypes.bfloat16
    C_ = np.asarray(inputs["confs"]).shape[1]
    emb = np.asarray(inputs["emb"], np.float32)
    tokens = np.asarray(inputs["tokens"])
    confs = np.asarray(inputs["confs"])

    p = {}
    p["emb"] = emb.astype(bf)

    def gp(name):
        return _perm_gates(np.asarray(inputs[name], np.float32))

    Wih0f, Wih0b = gp("Wih0f"), gp("Wih0b")
    b0f, b0b = gp("b0f"), gp("b0b")
    Whh0f, Whh0b = gp("Whh0f"), gp("Whh0b")
    Wih1f, Wih1b = gp("Wih1f"), gp("Wih1b")
    b1f, b1b = gp("b1f"), gp("b1b")
    Whh1f, Whh1b = gp("Whh1f"), gp("Whh1b")
    w1 = np.asarray(inputs["w1"], np.float32)
    bw1 = np.asarray(inputs["bw1"], np.float32)
    w2 = np.asarray(inputs["w2"], np.float32)
    bw2 = np.asarray(inputs["bw2"], np.float32)

    # W0s: x-proj chunks (128, 96, 128, 96); bias at row 95 of 96-chunks
    w0 = np.zeros((448, G4), np.float32)
    w0[0:128] = Wih0f.T[0:128]
    w0[128:200] = Wih0f.T[128:200]
    w0[223] = b0f
    w0[224:352] = Wih0b.T[0:128]
    w0[352:424] = Wih0b.T[128:200]
    w0[447] = b0b
    p["W0s"] = w0.astype(bf)

    def rec_stream(wf, wb):
        o = np.zeros((400, G4), np.float32)
        o[0:128] = wf.T[0:128]
        o[128:200] = wf.T[128:200]
        o[200:328] = wb.T[0:128]
        o[328:400] = wb.T[128:200]
        return o.astype(bf)

    p["Whh0s"] = rec_stream(Whh0f, Whh0b)
    p["Whh1s"] = rec_stream(Whh1f, Whh1b)
    p["W1sf"] = np.concatenate([Wih1f.T, b1f[None, :]], 0).astype(bf)
    p["W1sb"] = np.concatenate([Wih1b.T, b1b[None, :]], 0).astype(bf)

    # WUs: K = h1 feature, N = [U0 cols | U1 cols]
    w1T = w1.T  # [800, 400]
    wu = np.zeros((400, G4), np.float32)
    wu[:, 0:400] = w1T[0:400]
    wu[:, 400:800] = w1T[400:800]
    p["WUs"] = wu.astype(bf)
    p["bw1m"] = np.tile(bw1[None, :], (128, 1)).astype(bf)
    w2p = np.zeros((512, 4), np.float32)
    w2p[0:2 * H] = w2.T
    w2p[511] = bw2
    p["W2s"] = w2p.astype(bf)

    NGX = T // XG
    NP = BL * C_
    NPT = (NP + 127) // 128

    in_maps = []
    for c in range(n_cores):
        m = dict(p)
        bs = tokens[c * BL:(c + 1) * BL, 0:T]          # [BL, T]
        tf = np.zeros((128, NGX), np.int32)
        tb = np.zeros((128, NGX), np.int32)
        for g in range(NGX):
            for a in range(XG):
                tf[a * BL:(a + 1) * BL, g] = bs[:, XG * g + a]
                tb[a * BL:(a + 1) * BL, g] = bs[:, T - 1 - (XG * g + a)]
        m["tokf"] = tf
        m["tokb"] = tb
        cf = confs[c * BL:(c + 1) * BL]                 # [BL, C, 2]
        t0 = cf[:, :, 0].reshape(-1)                    # row-major (b, ci)
        t1 = cf[:, :, 1].reshape(-1)
        bidx = np.repeat(np.arange(BL), C_)
        ui0 = np.clip(t0, 0, T - 1) * BL + bidx
        ui1 = np.clip(t1, 0, T - 1) * BL + bidx
        um0 = (t0 >= 0).astype(np.float32)
        um1 = (t1 >= 0).astype(np.float32)

        def tile128(a, dt):
            o = np.zeros((NPT * 128,), dt)
            o[:a.shape[0]] = a
            return o.reshape(NPT, 128).T.copy()
        m["uidx0"] = tile128(ui0.astype(np.int32), np.int32)
        m["uidx1"] = tile128(ui1.astype(np.int32), np.int32)
        m["umask0"] = tile128(um0, np.float32)
        m["umask1"] = tile128(um1, np.float32)
        in_maps.append(m)
    return in_maps


_CACHE = {}


def _get_prog(T, n_cores, NPT):
    key = (T, n_cores, NPT)
    if key not in _CACHE:
        _CACHE[key] = build(T, n_cores, NPT)
    return _CACHE[key]


def kernel(**inputs):
    T = inputs["tokens"].shape[1]
    C_ = inputs["confs"].shape[1]
    n_cores = NCORE
    NP = BL * C_
    NPT = (NP + 127) // 128
    nc = _get_prog(T, n_cores, NPT)
    in_maps = prepare_inputs(inputs, T, n_cores)
    res = run_bass_kernel_spmd(nc, in_maps, list(range(n_cores)))
    outs = []
    for c in range(n_cores):
        o = res.results[c]["OUT"][:NP]          # [BL*C, 4] rows (b, ci)
        outs.append(o)
    return np.concatenate(outs, axis=0).astype(np.float32)


# revision 64
# speedup vs baseline: 1.2091x; 1.0336x over previous
"""Trainium2 Bass kernel for nn_BiLSTMNet (2-layer BiLSTM + pair-gather MLP).

v2 design (latency-focused):
- Data-parallel across 8 cores (16 sentences each), fwd+bwd fused per step
  (32 rows) via block-diagonal K in the recurrent matmul.
- h^T lhsT tiles live in 16-step SBUF *history* buffers; DRAM h stores are
  batched to 4 DMAs per 16 steps (vs 8/step in v1) - kills the HWDGE
  descriptor-generation bottleneck (625ns per DMA instruction, single queue).
- bf16 elementwise chain (sigmoids/tanh outputs, cell state, products) for
  DVE 2x mode; fp32 only in PSUM matmul accumulators.
- x embedding gathers via Pool/SWDGE; x^T staging via PE transposes (no
  DMA-transposes).
- bwd-direction stores / loads use reversed-block access patterns so h0T/h1T
  hold time-aligned [hf(t); hb(t)] and every DMA is a single descriptor run.
- U = h1 @ w1^T phase batched (4-chunk groups); MLP pair-gather via SWDGE.
"""
import sys
sys.path.insert(0, "/opt/trn_rl_repo")
import numpy as np
import ml_dtypes

import concourse.bass as bass
import concourse.tile as tile
from concourse import mybir, bacc
from concourse.bass_utils import run_bass_kernel_spmd
from concourse.masks import make_identity

BF16 = mybir.dt.bfloat16
F32 = mybir.dt.float32
I32 = mybir.dt.int32
AF = mybir.ActivationFunctionType
ALU = mybir.AluOpType

V, E, H, B, C = 32000, 200, 200, 128, 256
T_FULL = 512
BL = 16            # sentences per core
NB = 32            # rows per step (16 fwd + 16 bwd)
NCORE = 8
CHT = 4            # steps per PSUM chunk (4*32 = 128 slots)
XG = 8             # steps per x-gather group (2 chunks)
NSTG = 16          # steps per store / D-load group
G4 = 800           # 4*H gate width


def build(T, n_cores, NPT):
    NSLOT = T * BL
    NCH = T // CHT
    NGX = T // XG
    NGS = T // NSTG

    nc = bacc.Bacc("TRN2", target_bir_lowering=False, debug=False,
                   enable_asserts=True, num_devices=n_cores)

    def din(name, shape, dt):
        return nc.dram_tensor(name, shape, dt, kind="ExternalInput").ap()

    def dout(name, shape, dt):
        return nc.dram_tensor(name, shape, dt, kind="ExternalOutput").ap()

    emb = din("emb", [V, E], BF16)
    W0s = din("W0s", [448, G4], BF16)      # x-proj chunks (128,96,128,96), bias@row95
    Whh0s = din("Whh0s", [400, G4], BF16)  # rec chunks (128,72,128,72)
    W1sf = din("W1sf", [401, G4], BF16)    # L1 x-proj fwd (128,128,128,17)
    W1sb = din("W1sb", [401, G4], BF16)
    Whh1s = din("Whh1s", [400, G4], BF16)
    WUs = din("WUs", [400, G4], BF16)      # U chunks (128,72,128,72)
    W2s = din("W2s", [512, 4], BF16)
    tokf = din("tokf", [128, NGX], I32)
    tokb = din("tokb", [128, NGX], I32)
    uidx0 = din("uidx0", [128, NPT], I32)
    uidx1 = din("uidx1", [128, NPT], I32)
    umask0 = din("umask0", [128, NPT], F32)
    umask1 = din("umask1", [128, NPT], F32)
    bw1m = din("bw1m", [128, 2 * H], BF16)

    OUT = dout("OUT", [NPT * 128, 4], F32)
    import os
    DEBUG = os.environ.get("KDEBUG", "") == "1"
    if DEBUG:
        DXF = dout("DXF", [128, 224], F32)
        DCF = dout("DCF", [128, 256], F32)
        DCB1 = dout("DCB1", [96, 256], F32)
        DXG = dout("DXG", [128, 400], F32)
        DHH = dout("DHH", [32, 224], F32)
        DA0 = dout("DA0", [128, 512], F32)
        DSG = dout("DSG", [32, 600], F32)
        DS = dout("DS", [32, 400], F32)
        DG0 = dout("DG0", [128, 400], F32)
        DSS = dout("DSS", [128, 400], F32)
        DHM = dout("DHM", [128, 512], F32)
        DEX = dout("DEX", [128, 4], F32)

    # internal DRAM: time-aligned h^T ([hf(t);hb(t)] at col block t)
    h0T = nc.dram_tensor("h0T", [401, NSLOT], BF16).ap()   # row 400 = ones
    h1T = nc.dram_tensor("h1T", [400, NSLOT], BF16).ap()
    U0d = nc.dram_tensor("U0d", [NSLOT, 2 * H], BF16).ap()
    U1d = nc.dram_tensor("U1d", [NSLOT, 2 * H], BF16).ap()

    RS0 = [(0, 128), (128, 224), (224, 352), (352, 448)]   # W0s chunks
    RSH = [(0, 128), (128, 200), (200, 328), (328, 400)]   # Whh/WU chunks
    RS1 = [(0, 128), (128, 256), (256, 384), (384, 401)]   # L1 x chunks

    with tile.TileContext(nc) as tc:
        with tc.tile_pool(name="const", bufs=1) as cp, \
             tc.tile_pool(name="state", bufs=1) as sp:

            def load_chunks(src, rowsets, ncols, pfx):
                tiles = []
                for i, (r0, r1) in enumerate(rowsets):
                    t_ = cp.tile([r1 - r0, ncols], BF16, tag=f"{pfx}{i}",
                                 name=f"{pfx}{i}")
                    nc.sync.dma_start(out=t_[:], in_=src[r0:r1, :])
                    tiles.append(t_)
                return tiles

            W0t = load_chunks(W0s, RS0, G4, "W0")
            Whh0t = load_chunks(Whh0s, RSH, G4, "Wh0")
            W1ft = load_chunks(W1sf, RS1, G4, "W1f")
            W1bt = load_chunks(W1sb, RS1, G4, "W1b")
            Whh1t = load_chunks(Whh1s, RSH, G4, "Wh1")
            WUt = load_chunks(WUs, RSH, G4, "WU")
            W2t = load_chunks(W2s, [(0, 128), (128, 256), (256, 384), (384, 512)],
                              4, "W2")

            tokf_t = cp.tile([128, NGX], I32)
            tokb_t = cp.tile([128, NGX], I32)
            nc.sync.dma_start(out=tokf_t[:], in_=tokf[:])
            nc.sync.dma_start(out=tokb_t[:], in_=tokb[:])

            ones_row = cp.tile([1, NSLOT], BF16)
            nc.vector.memset(ones_row[:], 1.0)
            nc.sync.dma_start(out=h0T[400:401, :], in_=ones_row[:])

            ident32 = sp.tile([32, 32], BF16, name="ident32")
            ident128 = sp.tile([128, 128], BF16, name="ident128")
            make_identity(nc, ident32[:])
            make_identity(nc, ident128[:])

            # ---- persistent state
            # h^T history buffers (16 step slots x 32 cols; f cols 0:16, b 16:32)
            # double-buffered per 16-step group so the batched stores never
            # back-pressure the next group's writes
            AhB = [[sp.tile([128 if i % 2 == 0 else 72, 32 * NSTG], BF16,
                            tag=f"A{i}h{j}", name=f"A{i}h{j}") for i in range(4)]
                   for j in range(2)]
            S = sp.tile([NB, 2 * H], BF16, name="S")        # c | tanh(g)
            sigs = sp.tile([NB, 624], BF16, name="sigs")    # s(f),s(i) | s(o) | pad
            tcl = sp.tile([NB, 224], BF16, name="tcl")      # tanh(c) | pad
            prodt = sp.tile([NB, 2 * H], BF16, name="prodt")
            soT = sp.tile([128, 64], BF16, name="soT")      # s(o)^T staging
            tcT = sp.tile([128, 64], BF16, name="tcT")      # tanh(c)^T staging
            # x gather tiles + x^T lhsT (2-chunk groups, parity)
            xf = [sp.tile([128, 224], BF16, tag=f"xf{i}", name=f"xf{i}") for i in range(2)]
            xb = [sp.tile([128, 224], BF16, tag=f"xb{i}", name=f"xb{i}") for i in range(2)]
            Cf0 = [sp.tile([128, 256], BF16, tag=f"Cf0{i}", name=f"Cf0{i}") for i in range(2)]
            Cf1 = [sp.tile([96, 256], BF16, tag=f"Cf1{i}", name=f"Cf1{i}") for i in range(2)]
            Cb0 = [sp.tile([128, 256], BF16, tag=f"Cb0{i}", name=f"Cb0{i}") for i in range(2)]
            Cb1 = [sp.tile([96, 256], BF16, tag=f"Cb1{i}", name=f"Cb1{i}") for i in range(2)]
            # L1 lhsT tiles (4-chunk groups, parity); b-halves/f-halves stay 0
            Df = [[sp.tile([r1 - r0, 512], BF16, tag=f"Df{i}{j}", name=f"Df{i}{j}")
                   for i, (r0, r1) in enumerate(RS1)] for j in range(2)]
            Db = [[sp.tile([r1 - r0, 512], BF16, tag=f"Db{i}{j}", name=f"Db{i}{j}")
                   for i, (r0, r1) in enumerate(RS1)] for j in range(2)]

            for t_ in [S, sigs, tcl, prodt, soT, tcT] + AhB[0] + AhB[1] + xf + xb:
                nc.vector.memset(t_[:], 0.0)
            for j in range(2):
                for t_ in [Cf0[j], Cf1[j], Cb0[j], Cb1[j]] + Df[j] + Db[j]:
                    nc.vector.memset(t_[:], 0.0)
                # bias rows: 1.0 only in the direction's own column halves
                # (engine ops can't start at partition 95 -> use SBUF-SBUF DMA)
                ov = ones_row[:, 0:128].rearrange("p (a b) -> p a b", b=16)
                vf = Cf1[j][95:96, :].rearrange("p (a c b) -> p a c b", c=2, b=16)
                nc.sync.dma_start(out=vf[:, :, 0, :], in_=ov)
                vb = Cb1[j][95:96, :].rearrange("p (a c b) -> p a c b", c=2, b=16)
                nc.sync.dma_start(out=vb[:, :, 1, :], in_=ov)

            def ahf(bj, i):
                # f-half view [p, s, 16] of history i, buffer bj
                return AhB[bj][i][:].rearrange("p (s c b) -> p s c b", c=2, b=16)[:, :, 0, :]

            def ahb(bj, i):
                return AhB[bj][i][:].rearrange("p (s c b) -> p s c b", c=2, b=16)[:, :, 1, :]

            with tc.tile_pool(name="xps", bufs=2, space="PSUM") as xps, \
                 tc.tile_pool(name="tps", bufs=1, space="PSUM") as tps:

                # persistent PSUM staging tiles. Zero-region (bank) granularity
                # matters: a transpose's start=True marks its whole bank, so
                # readers of OTHER regions in that bank get serialized after
                # it. tc^T gets its own bank; so^T shares the x-stage bank
                # (written only once per 8 steps).
                psTc = tps.tile([128, 64], BF16, tag="psTc", name="psTc")
                px = tps.tile([128, 320], BF16, tag="px", name="px")

                xg_tiles = {}

                def alloc_xg(k):
                    # fi / g / o in separate banks: each gate group start=True
                    # marks its whole 2KB zero region, so groups can't share
                    xgfi = xps.tile([128, 400], F32, space="PSUM", tag="xgfi",
                                    name="xgfi", padded_shape=[128, 512])
                    xgg = xps.tile([128, 200], F32, space="PSUM", tag="xgg",
                                   name="xgg", padded_shape=[128, 512])
                    xgo = xps.tile([128, 200], F32, space="PSUM", tag="xgo",
                                   name="xgo", padded_shape=[128, 512])
                    xg_tiles[k] = (xgfi, xgg, xgo)
                    return xg_tiles[k]

                def emit_xg0_mms(k, i):
                    # i-th matmul (of 12) of chunk k: fi kc0-3, g kc0-3, o kc0-3
                    xgfi, xgg, xgo = xg_tiles[k]
                    par = (k // 2) % 2
                    cc = k % 2
                    Ct = [Cf0[par], Cf1[par], Cb0[par], Cb1[par]]
                    kc = i % 4
                    lt = Ct[kc][:, 128 * cc:128 * cc + 128]
                    if i < 4:
                        nc.tensor.matmul(xgfi[:, 0:400], lt, W0t[kc][:, 0:400],
                                         start=(kc == 0), stop=(kc == 3))
                    elif i < 8:
                        nc.tensor.matmul(xgg[:, 0:200], lt, W0t[kc][:, 400:600],
                                         start=(kc == 0), stop=(kc == 3))
                    else:
                        nc.tensor.matmul(xgo[:, 0:200], lt, W0t[kc][:, 600:800],
                                         start=(kc == 0), stop=(kc == 3))

                def emit_xg1_mms(k, i):
                    # i-th matmul (of 24) of chunk k: fi kc0-7, g kc0-7, o kc0-7
                    xgfi, xgg, xgo = xg_tiles[k]
                    par = (k // 4) % 2
                    cc = k % 4
                    Dt = Df[par] + Db[par]
                    kc = i % 8
                    lt = Dt[kc][:, 128 * cc:128 * cc + 128]
                    if i < 8:
                        nc.tensor.matmul(xgfi[:, 0:400], lt,
                                         (W1ft + W1bt)[kc][:, 0:400],
                                         start=(kc == 0), stop=(kc == 7))
                    elif i < 16:
                        nc.tensor.matmul(xgg[:, 0:200], lt,
                                         (W1ft + W1bt)[kc][:, 400:600],
                                         start=(kc == 0), stop=(kc == 7))
                    else:
                        nc.tensor.matmul(xgo[:, 0:200], lt,
                                         (W1ft + W1bt)[kc][:, 600:800],
                                         start=(kc == 0), stop=(kc == 7))

                def emit_gathers(g):
                    par = g % 2
                    nc.gpsimd.indirect_dma_start(
                        out=xf[par][:, 0:E], out_offset=None, in_=emb[:],
                        in_offset=bass.IndirectOffsetOnAxis(ap=tokf_t[:, g:g + 1], axis=0))
                    nc.gpsimd.indirect_dma_start(
                        out=xb[par][:, 0:E], out_offset=None, in_=emb[:],
                        in_offset=bass.IndirectOffsetOnAxis(ap=tokb_t[:, g:g + 1], axis=0))

                def emit_xpose(g):
                    # PE transposes + strided copies into C tiles for group g
                    par = g % 2
                    for (src, C0, C1, dve) in ((xf[par], Cf0[par], Cf1[par], True),
                                               (xb[par], Cb0[par], Cb1[par], False)):
                        half = 0 if dve else 1
                        nc.tensor.transpose(px[:, 0:128], src[:, 0:128], ident128[:])
                        nc.tensor.transpose(px[0:96, 128:256], src[:, 128:224],
                                            ident128[:])
                        d0 = C0[:].rearrange("p (a c b) -> p a c b", c=2, b=16)[:, :, half, :]
                        s0 = px[:, 0:128].rearrange("p (a b) -> p a b", b=16)
                        d1 = C1[0:72].rearrange("p (a c b) -> p a c b", c=2, b=16)[:, :, half, :]
                        s1 = px[0:72, 128:256].rearrange("p (a b) -> p a b", b=16)
                        # keep all x-copies on Act: DVE carries the step chain
                        nc.scalar.copy(d0, s0)
                        nc.scalar.copy(d1, s1)

                def emit_step(p, Whht):
                    k = p // CHT
                    r = (p % CHT) * NB
                    sp_ = (p - 1) % NSTG
                    xgfi, xgg, xgo = xg_tiles[k]
                    AhR = AhB[((p - 1) // NSTG) % 2]
                    # recurrent matmuls (accumulate onto x-proj)
                    for (t_, w0, w1) in ((xgfi, 0, 400), (xgg, 400, 600),
                                         (xgo, 600, 800)):
                        for ei, kc in enumerate((0, 1, 2, 3)):
                            nc.tensor.matmul(t_[r:r + NB, 0:w1 - w0],
                                             AhR[kc][:, 32 * sp_:32 * sp_ + 32],
                                             Whht[kc][:, w0:w1],
                                             start=False, stop=(ei == 3),
                                             skip_group_check=True,
                                             tile_position=(0, r))
                    # gate nonlinearities
                    nc.scalar.activation(sigs[:, 0:400], xgfi[r:r + NB, 0:400],
                                         AF.Sigmoid)
                    nc.scalar.activation(S[:, H:2 * H], xgg[r:r + NB, 0:200],
                                         AF.Tanh)
                    nc.scalar.activation(sigs[:, 400:600], xgo[r:r + NB, 0:200],
                                         AF.Sigmoid)
                    # c update: split so the first half only waits on sig(f,i)
                    nc.vector.tensor_mul(prodt[:, 0:H], sigs[:, 0:H], S[:, 0:H])
                    nc.vector.tensor_mul(prodt[:, H:2 * H], sigs[:, H:2 * H],
                                         S[:, H:2 * H])
                    nc.vector.tensor_add(S[:, 0:H], prodt[:, 0:H], prodt[:, H:2 * H])
                    nc.scalar.activation(tcl[:, 0:H], S[:, 0:H], AF.Tanh)

                def emit_step_tail(p):
                    # h^T = so^T * tc^T computed directly in transposed space.
                    # so^T lands in the px bank (stays clear of tclT's bank
                    # marking), is staged to SBUF by Act off the chain; the
                    # DVE muls then read tc^T straight from PSUM.
                    sn = p % NSTG
                    nc.tensor.transpose(px[:, 256:288], sigs[:, 400:528],
                                        ident32[:])
                    nc.tensor.transpose(px[0:96, 288:320], sigs[:, 528:624],
                                        ident32[:])
                    nc.vector.tensor_copy(soT[:, 0:32], px[:, 256:288])
                    nc.vector.tensor_copy(soT[0:72, 32:64], px[0:72, 288:320])
                    nc.tensor.transpose(psTc[:, 0:32], tcl[:, 0:128], ident32[:])
                    nc.tensor.transpose(psTc[0:96, 32:64], tcl[:, 128:224],
                                        ident32[:])
                    nc.vector.tensor_copy(tcT[:, 0:32], psTc[:, 0:32])
                    nc.vector.tensor_copy(tcT[0:72, 32:64], psTc[0:72, 32:64])
                    AhW = AhB[(p // NSTG) % 2]
                    nc.vector.tensor_mul(AhW[0][:, 32 * sn:32 * sn + 16],
                                         tcT[:, 0:16], soT[:, 0:16])
                    nc.vector.tensor_mul(AhW[2][:, 32 * sn + 16:32 * sn + 32],
                                         tcT[:, 16:32], soT[:, 16:32])
                    nc.vector.tensor_mul(AhW[1][:, 32 * sn:32 * sn + 16],
                                         tcT[0:72, 32:48], soT[0:72, 32:48])
                    nc.vector.tensor_mul(AhW[3][:, 32 * sn + 16:32 * sn + 32],
                                         tcT[0:72, 48:64], soT[0:72, 48:64])

                def emit_stores(G, hT, T_):
                    # batched h^T stores for 16-step group G
                    bj = G % 2
                    hTv = hT[:].rearrange("p (t b) -> p t b", b=16)
                    nc.sync.dma_start(out=hTv[0:128, G * NSTG:(G + 1) * NSTG, :],
                                      in_=ahf(bj, 0))
                    nc.sync.dma_start(out=hTv[128:200, G * NSTG:(G + 1) * NSTG, :],
                                      in_=ahf(bj, 1))
                    rb = T_ // NSTG - 1 - G
                    nc.sync.dma_start(out=hTv[200:328, rb * NSTG:(rb + 1) * NSTG, :],
                                      in_=ahb(bj, 2)[:, ::-1, :])
                    nc.sync.dma_start(out=hTv[328:400, rb * NSTG:(rb + 1) * NSTG, :],
                                      in_=ahb(bj, 3)[:, ::-1, :])

                def emit_dloads(G):
                    # L1 lhsT loads for 16-step group G (4 chunks)
                    par = G % 2
                    for ri, (r0, r1) in enumerate(RS1):
                        dst = Df[par][ri][:].rearrange(
                            "p (a c b) -> p a c b", c=2, b=16)[:, :, 0, :]
                        src = h0T[r0:r1, G * 256:(G + 1) * 256].rearrange(
                            "p (s b) -> p s b", b=16)
                        nc.sync.dma_start(out=dst, in_=src)
                    rb = NGS - 1 - G
                    for ri, (r0, r1) in enumerate(RS1):
                        dst = Db[par][ri][:].rearrange(
                            "p (a c b) -> p a c b", c=2, b=16)[:, :, 1, :]
                        src = h0T[r0:r1, rb * 256:(rb + 1) * 256].rearrange(
                            "p (s b) -> p s b", b=16)[:, ::-1, :]
                        nc.sync.dma_start(out=dst, in_=src)

                def reset_states():
                    nc.vector.memset(S[:], 0.0)
                    for t_ in AhB[0] + AhB[1]:
                        nc.vector.memset(t_[:], 0.0)

                # ================= layer 0 =================
                emit_gathers(0)
                emit_xpose(0)
                alloc_xg(0)
                for i in range(12):
                    emit_xg0_mms(0, i)
                if DEBUG:
                    dstg = sp.tile([128, 400], F32, name="dstg")
                    dstg2 = sp.tile([128, 224], F32, name="dstg2")
                    nc.vector.tensor_copy(dstg2[:], xf[0][:])
                    nc.sync.dma_start(out=DXF[:], in_=dstg2[:])
                    nc.vector.tensor_copy(dstg[:, 0:256], Cf0[0][:])
                    nc.sync.dma_start(out=DCF[:], in_=dstg[:, 0:256])
                    nc.vector.tensor_copy(dstg[0:96, 0:256], Cb1[0][:])
                    nc.sync.dma_start(out=DCB1[:], in_=dstg[0:96, 0:256])
                    nc.vector.tensor_copy(dstg[:], xg_tiles[0][0][:])
                    nc.sync.dma_start(out=DXG[:], in_=dstg[:])
                for G2 in range(NGX):
                    if G2 + 1 < NGX:
                        emit_gathers(G2 + 1)
                    for a in range(XG):
                        p = XG * G2 + a
                        emit_step(p, Whh0t)
                        if G2 + 1 < NGX and a == 2:
                            emit_xpose(G2 + 1)
                        k = p // CHT
                        if k + 1 < NCH:
                            if a % CHT == 0:
                                alloc_xg(k + 1)
                            for q in range(3):
                                emit_xg0_mms(k + 1, (a % CHT) * 3 + q)
                        emit_step_tail(p)
                        if a % CHT == CHT - 1:
                            xg_tiles.pop(k, None)
                        if p % NSTG == NSTG - 1:
                            emit_stores(p // NSTG, h0T, T)
                # ================= layer 1 =================
                reset_states()
                emit_dloads(0)
                alloc_xg(0)
                for i in range(24):
                    emit_xg1_mms(0, i)
                for G in range(NGS):
                    if G + 1 < NGS:
                        emit_dloads(G + 1)
                    for a in range(NSTG):
                        p = NSTG * G + a
                        emit_step(p, Whh1t)
                        k = p // CHT
                        if k + 1 < NCH:
                            if a % CHT == 0:
                                alloc_xg(k + 1)
                            i0 = (a % CHT) * 6
                            for i in range(i0, i0 + 6):
                                emit_xg1_mms(k + 1, i)
                        emit_step_tail(p)
                        if a % CHT == CHT - 1:
                            xg_tiles.pop(k, None)
                    emit_stores(G, h1T, T)

            # ================= U phase =================
            with tc.tile_pool(name="uw", bufs=2) as uw, \
                 tc.tile_pool(name="ups", bufs=2, space="PSUM") as ups:
                for G in range(NSLOT // 512):
                    ut = []
                    for ri, (r0, r1) in enumerate(RSH):
                        t_ = uw.tile([r1 - r0, 512], BF16, tag=f"UL{ri}",
                                     name=f"UL{ri}")
                        nc.sync.dma_start(out=t_[:],
                                          in_=h1T[r0:r1, 512 * G:512 * (G + 1)])
                        ut.append(t_)
                    ustg = uw.tile([128, 3200], BF16, tag="ustg", name="ustg")
                    for cc in range(4):
                        psu0 = ups.tile([128, 400], F32, space="PSUM", tag="psu0",
                                        name="psu0", padded_shape=[128, 512])
                        psu1 = ups.tile([128, 400], F32, space="PSUM", tag="psu1",
                                        name="psu1", padded_shape=[128, 512])
                        for kc in range(4):
                            lt = ut[kc][:, 128 * cc:128 * cc + 128]
                            nc.tensor.matmul(psu0[:], lt, WUt[kc][:, 0:400],
                                             start=(kc == 0), stop=(kc == 3))
                            nc.tensor.matmul(psu1[:], lt, WUt[kc][:, 400:800],
                                             start=(kc == 0), stop=(kc == 3))
                        nc.vector.tensor_copy(ustg[:, 800 * cc:800 * cc + 400],
                                              psu0[:])
                        nc.scalar.copy(ustg[:, 800 * cc + 400:800 * (cc + 1)],
                                       psu1[:])
                    sv = ustg[:].rearrange("p (c j) -> p c j", j=800)
                    d0 = U0d[512 * G:512 * (G + 1), :].rearrange(
                        "(c p) j -> p c j", p=128)
                    d1 = U1d[512 * G:512 * (G + 1), :].rearrange(
                        "(c p) j -> p c j", p=128)
                    nc.sync.dma_start(out=d0, in_=sv[:, :, 0:400])
                    nc.sync.dma_start(out=d1, in_=sv[:, :, 400:800])

            # ================= gather + MLP =================
            with tc.tile_pool(name="fw", bufs=2) as fw, \
                 tc.tile_pool(name="fc", bufs=1) as fc, \
                 tc.tile_pool(name="fps", bufs=2, space="PSUM") as fps:
                ui0 = fc.tile([128, NPT], I32)
                ui1 = fc.tile([128, NPT], I32)
                um0 = fc.tile([128, NPT], F32)
                um1 = fc.tile([128, NPT], F32)
                nc.sync.dma_start(out=ui0[:], in_=uidx0[:])
                nc.sync.dma_start(out=ui1[:], in_=uidx1[:])
                nc.sync.dma_start(out=um0[:], in_=umask0[:])
                nc.sync.dma_start(out=um1[:], in_=umask1[:])
                bwt = fc.tile([128, 2 * H], BF16, name="bwt")
                nc.sync.dma_start(out=bwt[:], in_=bw1m[:])
                hm = [fc.tile([128, 512], BF16, tag=f"hm{i}", name=f"hm{i}")
                      for i in range(2)]
                for t_ in hm:
                    nc.vector.memset(t_[:], 0.0)
                    nc.vector.memset(t_[:, 511:512], 1.0)
                otstg = fc.tile([128, 16], F32, name="otstg")
                for j in range(NPT):
                    par = j % 2
                    g0 = fw.tile([128, 2 * H], BF16, tag="g0", name="g0")
                    g1 = fw.tile([128, 2 * H], BF16, tag="g1", name="g1")
                    nc.gpsimd.indirect_dma_start(
                        out=g0[:], out_offset=None, in_=U0d[:],
                        in_offset=bass.IndirectOffsetOnAxis(ap=ui0[:, j:j + 1], axis=0))
                    nc.gpsimd.indirect_dma_start(
                        out=g1[:], out_offset=None, in_=U1d[:],
                        in_offset=bass.IndirectOffsetOnAxis(ap=ui1[:, j:j + 1], axis=0))
                    g1m = fw.tile([128, 2 * H], BF16, tag="g1m", name="g1m")
                    nc.vector.scalar_tensor_tensor(g1m[:], g1[:], um1[:, j:j + 1],
                                                   bwt[:], ALU.mult, ALU.add)
                    ssum = fw.tile([128, 2 * H], BF16, tag="ssum", name="ssum")
                    nc.vector.scalar_tensor_tensor(ssum[:], g0[:], um0[:, j:j + 1],
                                                   g1m[:], ALU.mult, ALU.add)
                    nc.scalar.activation(hm[par][:, 0:2 * H], ssum[:], AF.Tanh)
                    psl = fps.tile([128, 4], F32, space="PSUM", tag="psl", name="psl")
                    pst = fps.tile([128, 512], BF16, space="PSUM", tag="pst",
                                   name="pst")
                    for i in range(4):
                        nc.tensor.transpose(pst[:, 128 * i:128 * (i + 1)],
                                            hm[par][:, 128 * i:128 * (i + 1)],
                                            ident128[:])
                        hmT = fw.tile([128, 128], BF16, tag=f"hmT{i}", name=f"hmT{i}")
                        if i % 2 == 0:
                            nc.vector.tensor_copy(hmT[:], pst[:, 128 * i:128 * (i + 1)])
                        else:
                            nc.scalar.copy(hmT[:], pst[:, 128 * i:128 * (i + 1)])
                        nc.tensor.matmul(psl[:], hmT[:], W2t[i][:],
                                         start=(i == 0), stop=(i == 3))
                    ex = fw.tile([128, 4], F32, tag="ex", name="ex")
                    nc.scalar.activation(ex[:], psl[:], AF.Exp)
                    if DEBUG and j == 0:
                        dmt = fc.tile([128, 512], F32, name="dmt")
                        nc.vector.tensor_copy(dmt[:, 0:400], g0[:])
                        nc.sync.dma_start(out=DG0[:], in_=dmt[:, 0:400])
                        nc.vector.tensor_copy(dmt[:, 0:400], ssum[:])
                        nc.sync.dma_start(out=DSS[:], in_=dmt[:, 0:400])
                        nc.vector.tensor_copy(dmt[:, 0:512], hm[par][:])
                        nc.sync.dma_start(out=DHM[:], in_=dmt[:, 0:512])
                        nc.vector.tensor_copy(dmt[:, 0:4], ex[:])
                        nc.sync.dma_start(out=DEX[:], in_=dmt[:, 0:4])
                    sm = fw.tile([128, 1], F32, tag="sm", name="sm")
                    nc.vector.reduce_sum(sm[:], ex[:], axis=mybir.AxisListType.X)
                    rc = fw.tile([128, 1], F32, tag="rc", name="rc")
                    nc.vector.reciprocal(rc[:], sm[:])
                    jj = j % 4
                    nc.vector.tensor_scalar_mul(otstg[:, 4 * jj:4 * jj + 4],
                                                ex[:], rc[:, 0:1])
                    if DEBUG and j == 0:
                        dmt2 = fc.tile([128, 8], F32, name="dmt2")
                        nc.vector.tensor_copy(dmt2[:, 0:1], sm[:])
                        nc.vector.tensor_copy(dmt2[:, 1:2], rc[:])
                        nc.vector.tensor_copy(dmt2[:, 2:6], otstg[:, 0:4])
                        nc.sync.dma_start(out=DEX[:], in_=dmt2[:, 2:6])
                    if jj == 3 or j == NPT - 1:
                        nb = jj + 1
                        dst = OUT[128 * (j - jj):128 * (j + 1), :].rearrange(
                            "(c p) j -> p c j", p=128)
                        src = otstg[:, 0:4 * nb].rearrange("p (c j) -> p c j", j=4)
                        nc.sync.dma_start(out=dst, in_=src)
    nc.compile()
    return nc


# ---------------------------------------------------------------------------
# host-side preparation
# ---------------------------------------------------------------------------

def _perm_gates(w):
    """torch gate order (i,f,g,o) -> (f,i,g,o) along axis 0 (4H rows)."""
    Hq = w.shape[0] // 4
    i, f, g, o = (w[0:Hq], w[Hq:2 * Hq], w[2 * Hq:3 * Hq], w[3 * Hq:4 * Hq])
    return np.concatenate([f, i, g, o], axis=0)


def prepare_inputs(inputs, T, n_cores):
    bf = ml_dtypes.bfloat16
    C_ = np.asarray(inputs["confs"]).shape[1]
    emb = np.asarray(inputs["emb"], np.float32)
    tokens = np.asarray(inputs["tokens"])
    confs = np.asarray(inputs["confs"])

    p = {}
    p["emb"] = emb.astype(bf)

    def gp(name):
        return _perm_gates(np.asarray(inputs[name], np.float32))

    Wih0f, Wih0b = gp("Wih0f"), gp("Wih0b")
    b0f, b0b = gp("b0f"), gp("b0b")
    Whh0f, Whh0b = gp("Whh0f"), gp("Whh0b")
    Wih1f, Wih1b = gp("Wih1f"), gp("Wih1b")
    b1f, b1b = gp("b1f"), gp("b1b")
    Whh1f, Whh1b = gp("Whh1f"), gp("Whh1b")
    w1 = np.asarray(inputs["w1"], np.float32)
    bw1 = np.asarray(inputs["bw1"], np.float32)
    w2 = np.asarray(inputs["w2"], np.float32)
    bw2 = np.asarray(inputs["bw2"], np.float32)

    # W0s: x-proj chunks (128, 96, 128, 96); bias at row 95 of 96-chunks
    w0 = np.zeros((448, G4), np.float32)
    w0[0:128] = Wih0f.T[0:128]
    w0[128:200] = Wih0f.T[128:200]
    w0[223] = b0f
    w0[224:352] = Wih0b.T[0:128]
    w0[352:424] = Wih0b.T[128:200]
    w0[447] = b0b
    p["W0s"] = w0.astype(bf)

    def rec_stream(wf, wb):
        o = np.zeros((400, G4), np.float32)
        o[0:128] = wf.T[0:128]
        o[128:200] = wf.T[128:200]
        o[200:328] = wb.T[0:128]
        o[328:400] = wb.T[128:200]
        return o.astype(bf)

    p["Whh0s"] = rec_stream(Whh0f, Whh0b)
    p["Whh1s"] = rec_stream(Whh1f, Whh1b)
    p["W1sf"] = np.concatenate([Wih1f.T, b1f[None, :]], 0).astype(bf)
    p["W1sb"] = np.concatenate([Wih1b.T, b1b[None, :]], 0).astype(bf)

    # WUs: K = h1 feature, N = [U0 cols | U1 cols]
    w1T = w1.T  # [800, 400]
    wu = np.zeros((400, G4), np.float32)
    wu[:, 0:400] = w1T[0:400]
    wu[:, 400:800] = w1T[400:800]
    p["WUs"] = wu.astype(bf)
    p["bw1m"] = np.tile(bw1[None, :], (128, 1)).astype(bf)
    w2p = np.zeros((512, 4), np.float32)
    w2p[0:2 * H] = w2.T
    w2p[511] = bw2
    p["W2s"] = w2p.astype(bf)

    NGX = T // XG
    NP = BL * C_
    NPT = (NP + 127) // 128

    in_maps = []
    for c in range(n_cores):
        m = dict(p)
        bs = tokens[c * BL:(c + 1) * BL, 0:T]          # [BL, T]
        tf = np.zeros((128, NGX), np.int32)
        tb = np.zeros((128, NGX), np.int32)
        for g in range(NGX):
            for a in range(XG):
                tf[a * BL:(a + 1) * BL, g] = bs[:, XG * g + a]
                tb[a * BL:(a + 1) * BL, g] = bs[:, T - 1 - (XG * g + a)]
        m["tokf"] = tf
        m["tokb"] = tb
        cf = confs[c * BL:(c + 1) * BL]                 # [BL, C, 2]
        t0 = cf[:, :, 0].reshape(-1)                    # row-major (b, ci)
        t1 = cf[:, :, 1].reshape(-1)
        bidx = np.repeat(np.arange(BL), C_)
        ui0 = np.clip(t0, 0, T - 1) * BL + bidx
        ui1 = np.clip(t1, 0, T - 1) * BL + bidx
        um0 = (t0 >= 0).astype(np.float32)
        um1 = (t1 >= 0).astype(np.float32)

        def tile128(a, dt):
            o = np.zeros((NPT * 128,), dt)
            o[:a.shape[0]] = a
            return o.reshape(NPT, 128).T.copy()
        m["uidx0"] = tile128(ui0.astype(np.int32), np.int32)
        m["uidx1"] = tile128(ui1.astype(np.int32), np.int32)
        m["umask0"] = tile128(um0, np.float32)
        m["umask1"] = tile128(um1, np.float32)
        in_maps.append(m)
    return in_maps


_CACHE = {}


def _get_prog(T, n_cores, NPT):
    key = (T, n_cores, NPT)
    if key not in _CACHE:
        _CACHE[key] = build(T, n_cores, NPT)
    return _CACHE[key]


def kernel(**inputs):
    T = inputs["tokens"].shape[1]
    C_ = inputs["confs"].shape[1]
    n_cores = NCORE
    NP = BL * C_
    NPT = (NP + 127) // 128
    nc = _get_prog(T, n_cores, NPT)
    in_maps = prepare_inputs(inputs, T, n_cores)
    res = run_bass_kernel_spmd(nc, in_maps, list(range(n_cores)))
    outs = []
    for c in range(n_cores):
        o = res.results[c]["OUT"][:NP]          # [BL*C, 4] rows (b, ci)
        outs.append(o)
    return np.concatenate(outs, axis=0).astype(np.float32)


# revision 66
# speedup vs baseline: 1.2190x; 1.0082x over previous
"""Trainium2 Bass kernel for nn_BiLSTMNet (2-layer BiLSTM + pair-gather MLP).

v2 design (latency-focused):
- Data-parallel across 8 cores (16 sentences each), fwd+bwd fused per step
  (32 rows) via block-diagonal K in the recurrent matmul.
- h^T lhsT tiles live in 16-step SBUF *history* buffers; DRAM h stores are
  batched to 4 DMAs per 16 steps (vs 8/step in v1) - kills the HWDGE
  descriptor-generation bottleneck (625ns per DMA instruction, single queue).
- bf16 elementwise chain (sigmoids/tanh outputs, cell state, products) for
  DVE 2x mode; fp32 only in PSUM matmul accumulators.
- x embedding gathers via Pool/SWDGE; x^T staging via PE transposes (no
  DMA-transposes).
- bwd-direction stores / loads use reversed-block access patterns so h0T/h1T
  hold time-aligned [hf(t); hb(t)] and every DMA is a single descriptor run.
- U = h1 @ w1^T phase batched (4-chunk groups); MLP pair-gather via SWDGE.
"""
import sys
sys.path.insert(0, "/opt/trn_rl_repo")
import numpy as np
import ml_dtypes

import concourse.bass as bass
import concourse.tile as tile
from concourse import mybir, bacc
from concourse.bass_utils import run_bass_kernel_spmd
from concourse.masks import make_identity

BF16 = mybir.dt.bfloat16
F32 = mybir.dt.float32
I32 = mybir.dt.int32
AF = mybir.ActivationFunctionType
ALU = mybir.AluOpType

V, E, H, B, C = 32000, 200, 200, 128, 256
T_FULL = 512
BL = 16            # sentences per core
NB = 32            # rows per step (16 fwd + 16 bwd)
NCORE = 8
CHT = 4            # steps per PSUM chunk (4*32 = 128 slots)
XG = 8             # steps per x-gather group (2 chunks)
NSTG = 16          # steps per store / D-load group
G4 = 800           # 4*H gate width


def build(T, n_cores, NPT):
    NSLOT = T * BL
    NCH = T // CHT
    NGX = T // XG
    NGS = T // NSTG

    nc = bacc.Bacc("TRN2", target_bir_lowering=False, debug=False,
                   enable_asserts=True, num_devices=n_cores)

    def din(name, shape, dt):
        return nc.dram_tensor(name, shape, dt, kind="ExternalInput").ap()

    def dout(name, shape, dt):
        return nc.dram_tensor(name, shape, dt, kind="ExternalOutput").ap()

    emb = din("emb", [V, E], BF16)
    W0s = din("W0s", [448, G4], BF16)      # x-proj chunks (128,96,128,96), bias@row95
    Whh0s = din("Whh0s", [400, G4], BF16)  # rec chunks (128,72,128,72)
    W1sf = din("W1sf", [401, G4], BF16)    # L1 x-proj fwd (128,128,128,17)
    W1sb = din("W1sb", [401, G4], BF16)
    Whh1s = din("Whh1s", [400, G4], BF16)
    WUs = din("WUs", [400, G4], BF16)      # U chunks (128,72,128,72)
    W2s = din("W2s", [512, 4], BF16)
    tokf = din("tokf", [128, NGX], I32)
    tokb = din("tokb", [128, NGX], I32)
    uidx0 = din("uidx0", [128, NPT], I32)
    uidx1 = din("uidx1", [128, NPT], I32)
    umask0 = din("umask0", [128, NPT], F32)
    umask1 = din("umask1", [128, NPT], F32)
    bw1m = din("bw1m", [128, 2 * H], BF16)

    OUT = dout("OUT", [NPT * 128, 4], F32)
    import os
    DEBUG = os.environ.get("KDEBUG", "") == "1"
    if DEBUG:
        DXF = dout("DXF", [128, 224], F32)
        DCF = dout("DCF", [128, 256], F32)
        DCB1 = dout("DCB1", [96, 256], F32)
        DXG = dout("DXG", [128, 400], F32)
        DHH = dout("DHH", [32, 224], F32)
        DA0 = dout("DA0", [128, 512], F32)
        DSG = dout("DSG", [32, 600], F32)
        DS = dout("DS", [32, 400], F32)
        DG0 = dout("DG0", [128, 400], F32)
        DSS = dout("DSS", [128, 400], F32)
        DHM = dout("DHM", [128, 512], F32)
        DEX = dout("DEX", [128, 4], F32)

    # internal DRAM: time-aligned h^T ([hf(t);hb(t)] at col block t)
    h0T = nc.dram_tensor("h0T", [401, NSLOT], BF16).ap()   # row 400 = ones
    h1T = nc.dram_tensor("h1T", [400, NSLOT], BF16).ap()
    U0d = nc.dram_tensor("U0d", [NSLOT, 2 * H], BF16).ap()
    U1d = nc.dram_tensor("U1d", [NSLOT, 2 * H], BF16).ap()

    RS0 = [(0, 128), (128, 224), (224, 352), (352, 448)]   # W0s chunks
    RSH = [(0, 128), (128, 200), (200, 328), (328, 400)]   # Whh/WU chunks
    RS1 = [(0, 128), (128, 256), (256, 384), (384, 401)]   # L1 x chunks

    with tile.TileContext(nc) as tc:
        with tc.tile_pool(name="const", bufs=1) as cp, \
             tc.tile_pool(name="state", bufs=1) as sp:

            def load_chunks(src, rowsets, ncols, pfx):
                tiles = []
                for i, (r0, r1) in enumerate(rowsets):
                    t_ = cp.tile([r1 - r0, ncols], BF16, tag=f"{pfx}{i}",
                                 name=f"{pfx}{i}")
                    nc.sync.dma_start(out=t_[:], in_=src[r0:r1, :])
                    tiles.append(t_)
                return tiles

            W0t = load_chunks(W0s, RS0, G4, "W0")
            Whh0t = load_chunks(Whh0s, RSH, G4, "Wh0")
            W1ft = load_chunks(W1sf, RS1, G4, "W1f")
            W1bt = load_chunks(W1sb, RS1, G4, "W1b")
            Whh1t = load_chunks(Whh1s, RSH, G4, "Wh1")
            WUt = load_chunks(WUs, RSH, G4, "WU")
            W2t = load_chunks(W2s, [(0, 128), (128, 256), (256, 384), (384, 512)],
                              4, "W2")

            tokf_t = cp.tile([128, NGX], I32)
            tokb_t = cp.tile([128, NGX], I32)
            nc.sync.dma_start(out=tokf_t[:], in_=tokf[:])
            nc.sync.dma_start(out=tokb_t[:], in_=tokb[:])

            ones_row = cp.tile([1, NSLOT], BF16)
            nc.vector.memset(ones_row[:], 1.0)
            nc.sync.dma_start(out=h0T[400:401, :], in_=ones_row[:])

            ident32 = sp.tile([32, 32], BF16, name="ident32")
            ident128 = sp.tile([128, 128], BF16, name="ident128")
            make_identity(nc, ident32[:])
            make_identity(nc, ident128[:])

            # ---- persistent state
            # h^T history buffers (16 step slots x 32 cols; f cols 0:16, b 16:32)
            # double-buffered per 16-step group so the batched stores never
            # back-pressure the next group's writes
            AhB = [[sp.tile([128 if i % 2 == 0 else 72, 32 * NSTG], BF16,
                            tag=f"A{i}h{j}", name=f"A{i}h{j}") for i in range(4)]
                   for j in range(2)]
            S = sp.tile([NB, 2 * H], BF16, name="S")        # c | tanh(g)
            sigs = sp.tile([NB, 624], BF16, name="sigs")    # s(f),s(i) | s(o) | pad
            tcl = sp.tile([NB, 224], BF16, name="tcl")      # tanh(c) | pad
            prodt = sp.tile([NB, 2 * H], BF16, name="prodt")
            soT = sp.tile([128, 64], BF16, name="soT")      # s(o)^T staging
            tcT = sp.tile([128, 64], BF16, name="tcT")      # tanh(c)^T staging
            # x gather tiles + x^T lhsT (2-chunk groups, parity)
            xf = [sp.tile([128, 224], BF16, tag=f"xf{i}", name=f"xf{i}") for i in range(2)]
            xb = [sp.tile([128, 224], BF16, tag=f"xb{i}", name=f"xb{i}") for i in range(2)]
            Cf0 = [sp.tile([128, 256], BF16, tag=f"Cf0{i}", name=f"Cf0{i}") for i in range(2)]
            Cf1 = [sp.tile([96, 256], BF16, tag=f"Cf1{i}", name=f"Cf1{i}") for i in range(2)]
            Cb0 = [sp.tile([128, 256], BF16, tag=f"Cb0{i}", name=f"Cb0{i}") for i in range(2)]
            Cb1 = [sp.tile([96, 256], BF16, tag=f"Cb1{i}", name=f"Cb1{i}") for i in range(2)]
            # L1 lhsT tiles (4-chunk groups, parity); b-halves/f-halves stay 0
            Df = [[sp.tile([r1 - r0, 512], BF16, tag=f"Df{i}{j}", name=f"Df{i}{j}")
                   for i, (r0, r1) in enumerate(RS1)] for j in range(2)]
            Db = [[sp.tile([r1 - r0, 512], BF16, tag=f"Db{i}{j}", name=f"Db{i}{j}")
                   for i, (r0, r1) in enumerate(RS1)] for j in range(2)]

            for t_ in [S, sigs, tcl, prodt, soT, tcT] + AhB[0] + AhB[1] + xf + xb:
                nc.vector.memset(t_[:], 0.0)
            for j in range(2):
                for t_ in [Cf0[j], Cf1[j], Cb0[j], Cb1[j]] + Df[j] + Db[j]:
                    nc.vector.memset(t_[:], 0.0)
                # bias rows: 1.0 only in the direction's own column halves
                # (engine ops can't start at partition 95 -> use SBUF-SBUF DMA)
                ov = ones_row[:, 0:128].rearrange("p (a b) -> p a b", b=16)
                vf = Cf1[j][95:96, :].rearrange("p (a c b) -> p a c b", c=2, b=16)
                nc.sync.dma_start(out=vf[:, :, 0, :], in_=ov)
                vb = Cb1[j][95:96, :].rearrange("p (a c b) -> p a c b", c=2, b=16)
                nc.sync.dma_start(out=vb[:, :, 1, :], in_=ov)

            def ahf(bj, i):
                # f-half view [p, s, 16] of history i, buffer bj
                return AhB[bj][i][:].rearrange("p (s c b) -> p s c b", c=2, b=16)[:, :, 0, :]

            def ahb(bj, i):
                return AhB[bj][i][:].rearrange("p (s c b) -> p s c b", c=2, b=16)[:, :, 1, :]

            with tc.tile_pool(name="xps", bufs=2, space="PSUM") as xps, \
                 tc.tile_pool(name="tps", bufs=1, space="PSUM") as tps:

                # persistent PSUM staging tiles. Zero-region (bank) granularity
                # matters: a transpose's start=True marks its whole bank, so
                # readers of OTHER regions in that bank get serialized after
                # it. tc^T gets its own bank; so^T shares the x-stage bank
                # (written only once per 8 steps).
                psTc = tps.tile([128, 64], BF16, tag="psTc", name="psTc")
                px = tps.tile([128, 320], BF16, tag="px", name="px")

                xg_tiles = {}

                def alloc_xg(k):
                    # fi / g / o in separate banks: each gate group start=True
                    # marks its whole 2KB zero region, so groups can't share
                    xgfi = xps.tile([128, 400], F32, space="PSUM", tag="xgfi",
                                    name="xgfi", padded_shape=[128, 512])
                    xgg = xps.tile([128, 200], F32, space="PSUM", tag="xgg",
                                   name="xgg", padded_shape=[128, 512])
                    xgo = xps.tile([128, 200], F32, space="PSUM", tag="xgo",
                                   name="xgo", padded_shape=[128, 512])
                    xg_tiles[k] = (xgfi, xgg, xgo)
                    return xg_tiles[k]

                def emit_xg0_mms(k, i):
                    # i-th matmul (of 12) of chunk k: fi kc0-3, g kc0-3, o kc0-3
                    xgfi, xgg, xgo = xg_tiles[k]
                    par = (k // 2) % 2
                    cc = k % 2
                    Ct = [Cf0[par], Cf1[par], Cb0[par], Cb1[par]]
                    kc = i % 4
                    lt = Ct[kc][:, 128 * cc:128 * cc + 128]
                    if i < 4:
                        nc.tensor.matmul(xgfi[:, 0:400], lt, W0t[kc][:, 0:400],
                                         start=(kc == 0), stop=(kc == 3))
                    elif i < 8:
                        nc.tensor.matmul(xgg[:, 0:200], lt, W0t[kc][:, 400:600],
                                         start=(kc == 0), stop=(kc == 3))
                    else:
                        nc.tensor.matmul(xgo[:, 0:200], lt, W0t[kc][:, 600:800],
                                         start=(kc == 0), stop=(kc == 3))

                def emit_xg1_mms(k, i):
                    # i-th matmul (of 24) of chunk k: fi kc0-7, g kc0-7, o kc0-7
                    xgfi, xgg, xgo = xg_tiles[k]
                    par = (k // 4) % 2
                    cc = k % 4
                    Dt = Df[par] + Db[par]
                    kc = i % 8
                    lt = Dt[kc][:, 128 * cc:128 * cc + 128]
                    if i < 8:
                        nc.tensor.matmul(xgfi[:, 0:400], lt,
                                         (W1ft + W1bt)[kc][:, 0:400],
                                         start=(kc == 0), stop=(kc == 7))
                    elif i < 16:
                        nc.tensor.matmul(xgg[:, 0:200], lt,
                                         (W1ft + W1bt)[kc][:, 400:600],
                                         start=(kc == 0), stop=(kc == 7))
                    else:
                        nc.tensor.matmul(xgo[:, 0:200], lt,
                                         (W1ft + W1bt)[kc][:, 600:800],
                                         start=(kc == 0), stop=(kc == 7))

                def emit_gathers(g):
                    par = g % 2
                    nc.gpsimd.indirect_dma_start(
                        out=xf[par][:, 0:E], out_offset=None, in_=emb[:],
                        in_offset=bass.IndirectOffsetOnAxis(ap=tokf_t[:, g:g + 1], axis=0))
                    nc.gpsimd.indirect_dma_start(
                        out=xb[par][:, 0:E], out_offset=None, in_=emb[:],
                        in_offset=bass.IndirectOffsetOnAxis(ap=tokb_t[:, g:g + 1], axis=0))

                def emit_xpose(g):
                    # PE transposes + strided copies into C tiles for group g
                    par = g % 2
                    for (src, C0, C1, dve) in ((xf[par], Cf0[par], Cf1[par], True),
                                               (xb[par], Cb0[par], Cb1[par], False)):
                        half = 0 if dve else 1
                        nc.tensor.transpose(px[:, 0:128], src[:, 0:128], ident128[:])
                        nc.tensor.transpose(px[0:96, 128:256], src[:, 128:224],
                                            ident128[:])
                        d0 = C0[:].rearrange("p (a c b) -> p a c b", c=2, b=16)[:, :, half, :]
                        s0 = px[:, 0:128].rearrange("p (a b) -> p a b", b=16)
                        d1 = C1[0:72].rearrange("p (a c b) -> p a c b", c=2, b=16)[:, :, half, :]
                        s1 = px[0:72, 128:256].rearrange("p (a b) -> p a b", b=16)
                        # keep all x-copies on Act: DVE carries the step chain
                        nc.scalar.copy(d0, s0)
                        nc.scalar.copy(d1, s1)

                def emit_step(p, Whht):
                    k = p // CHT
                    r = (p % CHT) * NB
                    sp_ = (p - 1) % NSTG
                    xgfi, xgg, xgo = xg_tiles[k]
                    AhR = AhB[((p - 1) // NSTG) % 2]
                    # recurrent matmuls (accumulate onto x-proj)
                    for (t_, w0, w1) in ((xgfi, 0, 400), (xgg, 400, 600),
                                         (xgo, 600, 800)):
                        for ei, kc in enumerate((0, 2, 1, 3)):
                            nc.tensor.matmul(t_[r:r + NB, 0:w1 - w0],
                                             AhR[kc][:, 32 * sp_:32 * sp_ + 32],
                                             Whht[kc][:, w0:w1],
                                             start=False, stop=(ei == 3),
                                             skip_group_check=True,
                                             tile_position=(0, r))
                    # gate nonlinearities
                    nc.scalar.activation(sigs[:, 0:400], xgfi[r:r + NB, 0:400],
                                         AF.Sigmoid)
                    nc.scalar.activation(S[:, H:2 * H], xgg[r:r + NB, 0:200],
                                         AF.Tanh)
                    nc.scalar.activation(sigs[:, 400:600], xgo[r:r + NB, 0:200],
                                         AF.Sigmoid)
                    # c update: split so the first half only waits on sig(f,i)
                    nc.vector.tensor_mul(prodt[:, 0:H], sigs[:, 0:H], S[:, 0:H])
                    nc.vector.tensor_mul(prodt[:, H:2 * H], sigs[:, H:2 * H],
                                         S[:, H:2 * H])
                    nc.vector.tensor_add(S[:, 0:H], prodt[:, 0:H], prodt[:, H:2 * H])
                    nc.scalar.activation(tcl[:, 0:H], S[:, 0:H], AF.Tanh)

                def emit_step_tail(p):
                    # h^T = so^T * tc^T computed directly in transposed space.
                    # so^T lands in the px bank (stays clear of tclT's bank
                    # marking), is staged to SBUF by Act off the chain; the
                    # DVE muls then read tc^T straight from PSUM.
                    sn = p % NSTG
                    nc.tensor.transpose(px[:, 256:288], sigs[:, 400:528],
                                        ident32[:])
                    nc.tensor.transpose(px[0:96, 288:320], sigs[:, 528:624],
                                        ident32[:])
                    nc.vector.tensor_copy(soT[:, 0:32], px[:, 256:288])
                    nc.vector.tensor_copy(soT[0:72, 32:64], px[0:72, 288:320])
                    nc.tensor.transpose(psTc[:, 0:32], tcl[:, 0:128], ident32[:])
                    nc.tensor.transpose(psTc[0:96, 32:64], tcl[:, 128:224],
                                        ident32[:])
                    AhW = AhB[(p // NSTG) % 2]
                    nc.vector.tensor_copy(tcT[:, 0:32], psTc[:, 0:32])
                    nc.vector.tensor_mul(AhW[0][:, 32 * sn:32 * sn + 16],
                                         tcT[:, 0:16], soT[:, 0:16])
                    nc.vector.tensor_mul(AhW[2][:, 32 * sn + 16:32 * sn + 32],
                                         tcT[:, 16:32], soT[:, 16:32])
                    nc.vector.tensor_copy(tcT[0:72, 32:64], psTc[0:72, 32:64])
                    nc.vector.tensor_mul(AhW[1][:, 32 * sn:32 * sn + 16],
                                         tcT[0:72, 32:48], soT[0:72, 32:48])
                    nc.vector.tensor_mul(AhW[3][:, 32 * sn + 16:32 * sn + 32],
                                         tcT[0:72, 48:64], soT[0:72, 48:64])

                def emit_stores(G, hT, T_):
                    # batched h^T stores for 16-step group G
                    bj = G % 2
                    hTv = hT[:].rearrange("p (t b) -> p t b", b=16)
                    nc.sync.dma_start(out=hTv[0:128, G * NSTG:(G + 1) * NSTG, :],
                                      in_=ahf(bj, 0))
                    nc.sync.dma_start(out=hTv[128:200, G * NSTG:(G + 1) * NSTG, :],
                                      in_=ahf(bj, 1))
                    rb = T_ // NSTG - 1 - G
                    nc.sync.dma_start(out=hTv[200:328, rb * NSTG:(rb + 1) * NSTG, :],
                                      in_=ahb(bj, 2)[:, ::-1, :])
                    nc.sync.dma_start(out=hTv[328:400, rb * NSTG:(rb + 1) * NSTG, :],
                                      in_=ahb(bj, 3)[:, ::-1, :])

                def emit_dloads(G):
                    # L1 lhsT loads for 16-step group G (4 chunks)
                    par = G % 2
                    for ri, (r0, r1) in enumerate(RS1):
                        dst = Df[par][ri][:].rearrange(
                            "p (a c b) -> p a c b", c=2, b=16)[:, :, 0, :]
                        src = h0T[r0:r1, G * 256:(G + 1) * 256].rearrange(
                            "p (s b) -> p s b", b=16)
                        nc.sync.dma_start(out=dst, in_=src)
                    rb = NGS - 1 - G
                    for ri, (r0, r1) in enumerate(RS1):
                        dst = Db[par][ri][:].rearrange(
                            "p (a c b) -> p a c b", c=2, b=16)[:, :, 1, :]
                        src = h0T[r0:r1, rb * 256:(rb + 1) * 256].rearrange(
                            "p (s b) -> p s b", b=16)[:, ::-1, :]
                        nc.sync.dma_start(out=dst, in_=src)

                def reset_states():
                    nc.vector.memset(S[:], 0.0)
                    for t_ in AhB[0] + AhB[1]:
                        nc.vector.memset(t_[:], 0.0)

                # ================= layer 0 =================
                emit_gathers(0)
                emit_xpose(0)
                alloc_xg(0)
                for i in range(12):
                    emit_xg0_mms(0, i)
                if DEBUG:
                    dstg = sp.tile([128, 400], F32, name="dstg")
                    dstg2 = sp.tile([128, 224], F32, name="dstg2")
                    nc.vector.tensor_copy(dstg2[:], xf[0][:])
                    nc.sync.dma_start(out=DXF[:], in_=dstg2[:])
                    nc.vector.tensor_copy(dstg[:, 0:256], Cf0[0][:])
                    nc.sync.dma_start(out=DCF[:], in_=dstg[:, 0:256])
                    nc.vector.tensor_copy(dstg[0:96, 0:256], Cb1[0][:])
                    nc.sync.dma_start(out=DCB1[:], in_=dstg[0:96, 0:256])
                    nc.vector.tensor_copy(dstg[:], xg_tiles[0][0][:])
                    nc.sync.dma_start(out=DXG[:], in_=dstg[:])
                for G2 in range(NGX):
                    if G2 + 1 < NGX:
                        emit_gathers(G2 + 1)
                    for a in range(XG):
                        p = XG * G2 + a
                        emit_step(p, Whh0t)
                        if G2 + 1 < NGX and a == 2:
                            emit_xpose(G2 + 1)
                        k = p // CHT
                        if k + 1 < NCH:
                            if a % CHT == 0:
                                alloc_xg(k + 1)
                            for q in range(3):
                                emit_xg0_mms(k + 1, (a % CHT) * 3 + q)
                        emit_step_tail(p)
                        if a % CHT == CHT - 1:
                            xg_tiles.pop(k, None)
                        if p % NSTG == NSTG - 1:
                            emit_stores(p // NSTG, h0T, T)
                # ================= layer 1 =================
                reset_states()
                emit_dloads(0)
                alloc_xg(0)
                for i in range(24):
                    emit_xg1_mms(0, i)
                for G in range(NGS):
                    if G + 1 < NGS:
                        emit_dloads(G + 1)
                    for a in range(NSTG):
                        p = NSTG * G + a
                        emit_step(p, Whh1t)
                        k = p // CHT
                        if k + 1 < NCH:
                            if a % CHT == 0:
                                alloc_xg(k + 1)
                            i0 = (a % CHT) * 6
                            for i in range(i0, i0 + 6):
                                emit_xg1_mms(k + 1, i)
                        emit_step_tail(p)
                        if a % CHT == CHT - 1:
                            xg_tiles.pop(k, None)
                    emit_stores(G, h1T, T)

            # ================= U phase =================
            with tc.tile_pool(name="uw", bufs=2) as uw, \
                 tc.tile_pool(name="ups", bufs=2, space="PSUM") as ups:
                for G in range(NSLOT // 512):
                    ut = []
                    for ri, (r0, r1) in enumerate(RSH):
                        t_ = uw.tile([r1 - r0, 512], BF16, tag=f"UL{ri}",
                                     name=f"UL{ri}")
                        nc.sync.dma_start(out=t_[:],
                                          in_=h1T[r0:r1, 512 * G:512 * (G + 1)])
                        ut.append(t_)
                    ustg = uw.tile([128, 3200], BF16, tag="ustg", name="ustg")
                    for cc in range(4):
                        psu0 = ups.tile([128, 400], F32, space="PSUM", tag="psu0",
                                        name="psu0", padded_shape=[128, 512])
                        psu1 = ups.tile([128, 400], F32, space="PSUM", tag="psu1",
                                        name="psu1", padded_shape=[128, 512])
                        for kc in range(4):
                            lt = ut[kc][:, 128 * cc:128 * cc + 128]
                            nc.tensor.matmul(psu0[:], lt, WUt[kc][:, 0:400],
                                             start=(kc == 0), stop=(kc == 3))
                            nc.tensor.matmul(psu1[:], lt, WUt[kc][:, 400:800],
                                             start=(kc == 0), stop=(kc == 3))
                        nc.vector.tensor_copy(ustg[:, 800 * cc:800 * cc + 400],
                                              psu0[:])
                        nc.scalar.copy(ustg[:, 800 * cc + 400:800 * (cc + 1)],
                                       psu1[:])
                    sv = ustg[:].rearrange("p (c j) -> p c j", j=800)
                    d0 = U0d[512 * G:512 * (G + 1), :].rearrange(
                        "(c p) j -> p c j", p=128)
                    d1 = U1d[512 * G:512 * (G + 1), :].rearrange(
                        "(c p) j -> p c j", p=128)
                    nc.sync.dma_start(out=d0, in_=sv[:, :, 0:400])
                    nc.sync.dma_start(out=d1, in_=sv[:, :, 400:800])

            # ================= gather + MLP =================
            with tc.tile_pool(name="fw", bufs=2) as fw, \
                 tc.tile_pool(name="fc", bufs=1) as fc, \
                 tc.tile_pool(name="fps", bufs=2, space="PSUM") as fps:
                ui0 = fc.tile([128, NPT], I32)
                ui1 = fc.tile([128, NPT], I32)
                um0 = fc.tile([128, NPT], F32)
                um1 = fc.tile([128, NPT], F32)
                nc.sync.dma_start(out=ui0[:], in_=uidx0[:])
                nc.sync.dma_start(out=ui1[:], in_=uidx1[:])
                nc.sync.dma_start(out=um0[:], in_=umask0[:])
                nc.sync.dma_start(out=um1[:], in_=umask1[:])
                bwt = fc.tile([128, 2 * H], BF16, name="bwt")
                nc.sync.dma_start(out=bwt[:], in_=bw1m[:])
                hm = [fc.tile([128, 512], BF16, tag=f"hm{i}", name=f"hm{i}")
                      for i in range(2)]
                for t_ in hm:
                    nc.vector.memset(t_[:], 0.0)
                    nc.vector.memset(t_[:, 511:512], 1.0)
                otstg = fc.tile([128, 16], F32, name="otstg")
                for j in range(NPT):
                    par = j % 2
                    g0 = fw.tile([128, 2 * H], BF16, tag="g0", name="g0")
                    g1 = fw.tile([128, 2 * H], BF16, tag="g1", name="g1")
                    nc.gpsimd.indirect_dma_start(
                        out=g0[:], out_offset=None, in_=U0d[:],
                        in_offset=bass.IndirectOffsetOnAxis(ap=ui0[:, j:j + 1], axis=0))
                    nc.gpsimd.indirect_dma_start(
                        out=g1[:], out_offset=None, in_=U1d[:],
                        in_offset=bass.IndirectOffsetOnAxis(ap=ui1[:, j:j + 1], axis=0))
                    g1m = fw.tile([128, 2 * H], BF16, tag="g1m", name="g1m")
                    nc.vector.scalar_tensor_tensor(g1m[:], g1[:], um1[:, j:j + 1],
                                                   bwt[:], ALU.mult, ALU.add)
                    ssum = fw.tile([128, 2 * H], BF16, tag="ssum", name="ssum")
                    nc.vector.scalar_tensor_tensor(ssum[:], g0[:], um0[:, j:j + 1],
                                                   g1m[:], ALU.mult, ALU.add)
                    nc.scalar.activation(hm[par][:, 0:2 * H], ssum[:], AF.Tanh)
                    psl = fps.tile([128, 4], F32, space="PSUM", tag="psl", name="psl")
                    pst = fps.tile([128, 512], BF16, space="PSUM", tag="pst",
                                   name="pst")
                    for i in range(4):
                        nc.tensor.transpose(pst[:, 128 * i:128 * (i + 1)],
                                            hm[par][:, 128 * i:128 * (i + 1)],
                                            ident128[:])
                        hmT = fw.tile([128, 128], BF16, tag=f"hmT{i}", name=f"hmT{i}")
                        if i % 2 == 0:
                            nc.vector.tensor_copy(hmT[:], pst[:, 128 * i:128 * (i + 1)])
                        else:
                            nc.scalar.copy(hmT[:], pst[:, 128 * i:128 * (i + 1)])
                        nc.tensor.matmul(psl[:], hmT[:], W2t[i][:],
                                         start=(i == 0), stop=(i == 3))
                    ex = fw.tile([128, 4], F32, tag="ex", name="ex")
                    nc.scalar.activation(ex[:], psl[:], AF.Exp)
                    if DEBUG and j == 0:
                        dmt = fc.tile([128, 512], F32, name="dmt")
                        nc.vector.tensor_copy(dmt[:, 0:400], g0[:])
                        nc.sync.dma_start(out=DG0[:], in_=dmt[:, 0:400])
                        nc.vector.tensor_copy(dmt[:, 0:400], ssum[:])
                        nc.sync.dma_start(out=DSS[:], in_=dmt[:, 0:400])
                        nc.vector.tensor_copy(dmt[:, 0:512], hm[par][:])
                        nc.sync.dma_start(out=DHM[:], in_=dmt[:, 0:512])
                        nc.vector.tensor_copy(dmt[:, 0:4], ex[:])
                        nc.sync.dma_start(out=DEX[:], in_=dmt[:, 0:4])
                    sm = fw.tile([128, 1], F32, tag="sm", name="sm")
                    nc.vector.reduce_sum(sm[:], ex[:], axis=mybir.AxisListType.X)
                    rc = fw.tile([128, 1], F32, tag="rc", name="rc")
                    nc.vector.reciprocal(rc[:], sm[:])
                    jj = j % 4
                    nc.vector.tensor_scalar_mul(otstg[:, 4 * jj:4 * jj + 4],
                                                ex[:], rc[:, 0:1])
                    if DEBUG and j == 0:
                        dmt2 = fc.tile([128, 8], F32, name="dmt2")
                        nc.vector.tensor_copy(dmt2[:, 0:1], sm[:])
                        nc.vector.tensor_copy(dmt2[:, 1:2], rc[:])
                        nc.vector.tensor_copy(dmt2[:, 2:6], otstg[:, 0:4])
                        nc.sync.dma_start(out=DEX[:], in_=dmt2[:, 2:6])
                    if jj == 3 or j == NPT - 1:
                        nb = jj + 1
                        dst = OUT[128 * (j - jj):128 * (j + 1), :].rearrange(
                            "(c p) j -> p c j", p=128)
                        src = otstg[:, 0:4 * nb].rearrange("p (c j) -> p c j", j=4)
                        nc.sync.dma_start(out=dst, in_=src)
    nc.compile()
    return nc


# ---------------------------------------------------------------------------
# host-side preparation
# ---------------------------------------------------------------------------

def _perm_gates(w):
    """torch gate order (i,f,g,o) -> (f,i,g,o) along axis 0 (4H rows)."""
    Hq = w.shape[0] // 4
    i, f, g, o = (w[0:Hq], w[Hq:2 * Hq], w[2 * Hq:3 * Hq], w[3 * Hq:4 * Hq])
    return np.concatenate([f, i, g, o], axis=0)


def prepare_inputs(inputs, T, n_cores):
    bf = ml_dtypes.bfloat16
    C_ = np.asarray(inputs["confs"]).shape[1]
    emb = np.asarray(inputs["emb"], np.float32)
    tokens = np.asarray(inputs["tokens"])
    confs = np.asarray(inputs["confs"])

    p = {}
    p["emb"] = emb.astype(bf)

    def gp(name):
        return _perm_gates(np.asarray(inputs[name], np.float32))

    Wih0f, Wih0b = gp("Wih0f"), gp("Wih0b")
    b0f, b0b = gp("b0f"), gp("b0b")
    Whh0f, Whh0b = gp("Whh0f"), gp("Whh0b")
    Wih1f, Wih1b = gp("Wih1f"), gp("Wih1b")
    b1f, b1b = gp("b1f"), gp("b1b")
    Whh1f, Whh1b = gp("Whh1f"), gp("Whh1b")
    w1 = np.asarray(inputs["w1"], np.float32)
    bw1 = np.asarray(inputs["bw1"], np.float32)
    w2 = np.asarray(inputs["w2"], np.float32)
    bw2 = np.asarray(inputs["bw2"], np.float32)

    # W0s: x-proj chunks (128, 96, 128, 96); bias at row 95 of 96-chunks
    w0 = np.zeros((448, G4), np.float32)
    w0[0:128] = Wih0f.T[0:128]
    w0[128:200] = Wih0f.T[128:200]
    w0[223] = b0f
    w0[224:352] = Wih0b.T[0:128]
    w0[352:424] = Wih0b.T[128:200]
    w0[447] = b0b
    p["W0s"] = w0.astype(bf)

    def rec_stream(wf, wb):
        o = np.zeros((400, G4), np.float32)
        o[0:128] = wf.T[0:128]
        o[128:200] = wf.T[128:200]
        o[200:328] = wb.T[0:128]
        o[328:400] = wb.T[128:200]
        return o.astype(bf)

    p["Whh0s"] = rec_stream(Whh0f, Whh0b)
    p["Whh1s"] = rec_stream(Whh1f, Whh1b)
    p["W1sf"] = np.concatenate([Wih1f.T, b1f[None, :]], 0).astype(bf)
    p["W1sb"] = np.concatenate([Wih1b.T, b1b[None, :]], 0).astype(bf)

    # WUs: K = h1 feature, N = [U0 cols | U1 cols]
    w1T = w1.T  # [800, 400]
    wu = np.zeros((400, G4), np.float32)
    wu[:, 0:400] = w1T[0:400]
    wu[:, 400:800] = w1T[400:800]
    p["WUs"] = wu.astype(bf)
    p["bw1m"] = np.tile(bw1[None, :], (128, 1)).astype(bf)
    w2p = np.zeros((512, 4), np.float32)
    w2p[0:2 * H] = w2.T
    w2p[511] = bw2
    p["W2s"] = w2p.astype(bf)

    NGX = T // XG
    NP = BL * C_
    NPT = (NP + 127) // 128

    in_maps = []
    for c in range(n_cores):
        m = dict(p)
        bs = tokens[c * BL:(c + 1) * BL, 0:T]          # [BL, T]
        tf = np.zeros((128, NGX), np.int32)
        tb = np.zeros((128, NGX), np.int32)
        for g in range(NGX):
            for a in range(XG):
                tf[a * BL:(a + 1) * BL, g] = bs[:, XG * g + a]
                tb[a * BL:(a + 1) * BL, g] = bs[:, T - 1 - (XG * g + a)]
        m["tokf"] = tf
        m["tokb"] = tb
        cf = confs[c * BL:(c + 1) * BL]                 # [BL, C, 2]
        t0 = cf[:, :, 0].reshape(-1)                    # row-major (b, ci)
        t1 = cf[:, :, 1].reshape(-1)
        bidx = np.repeat(np.arange(BL), C_)
        ui0 = np.clip(t0, 0, T - 1) * BL + bidx
        ui1 = np.clip(t1, 0, T - 1) * BL + bidx
        um0 = (t0 >= 0).astype(np.float32)
        um1 = (t1 >= 0).astype(np.float32)

        def tile128(a, dt):
            o = np.zeros((NPT * 128,), dt)
            o[:a.shape[0]] = a
            return o.reshape(NPT, 128).T.copy()
        m["uidx0"] = tile128(ui0.astype(np.int32), np.int32)
        m["uidx1"] = tile128(ui1.astype(np.int32), np.int32)
        m["umask0"] = tile128(um0, np.float32)
        m["umask1"] = tile128(um1, np.float32)
        in_maps.append(m)
    return in_maps


_CACHE = {}


def _get_prog(T, n_cores, NPT):
    key = (T, n_cores, NPT)
    if key not in _CACHE:
        _CACHE[key] = build(T, n_cores, NPT)
    return _CACHE[key]


def kernel(**inputs):
    T = inputs["tokens"].shape[1]
    C_ = inputs["confs"].shape[1]
    n_cores = NCORE
    NP = BL * C_
    NPT = (NP + 127) // 128
    nc = _get_prog(T, n_cores, NPT)
    in_maps = prepare_inputs(inputs, T, n_cores)
    res = run_bass_kernel_spmd(nc, in_maps, list(range(n_cores)))
    outs = []
    for c in range(n_cores):
        o = res.results[c]["OUT"][:NP]          # [BL*C, 4] rows (b, ci)
        outs.append(o)
    return np.concatenate(outs, axis=0).astype(np.float32)
